# revision 37
# baseline (speedup 1.0000x reference)
"""Trainium2 Bass kernel for nn_DiffusionConv (two-direction GCN conv + relu).

out = relu(gcn(x, W_fwd; src->dst) + gcn(x, W_bwd; dst->src))

Algorithm (validated against the jax reference in numpy):
  gcn(x, W; edges) = D^-1/2 (A + I) D^-1/2 x W  with D = indegree+1.
  The weight GEMM commutes with aggregation, so the device aggregates scaled
  features u = dinv * x over edges (gather rows by source, reduce into
  destination tiles), applies W to each aggregated 128-row tile, scales by
  destination dinv, adds bias, relu.

Device mapping (one SPMD program on 8 cores):
  - nodes are permuted into tiles of 128 slots; tiles sharded across cores,
    processed in groups of 8 tiles per core.
  - per (group, stream), in-edges are chunked into groups of 128; batched
    dma_gather (3072 rows per call -- large calls amortize the ~1us SWDGE
    fixed cost, and keeping total DMA-instruction count low avoids a HW
    cliff; 4 SWDGE queues aligned to the 8 round-robin DMASW sem lanes)
    pulls source rows G [128e, 128f] from the fp16 table in DRAM.
  - M[e, r] = (dst_slot[e] == r) built on DVE with one is_equal against a
    precomputed iota matrix; PE matmul agg_T += G.T @ M accumulates in PSUM.
    M is 64 columns wide: each (stream, tile) edge section is ordered
    [slots 0-63 | slots 64-127] (pos stored mod 64, per-half chunk padding)
    and the matmuls target the matching PSUM column half -- halves both the
    DVE compare work and the PE matmul width.
  - self-loop terms use one direct DMA per group of the slot-permuted u
    tables (stored transposed [128, T, 128] in DRAM so the group slice is
    per-partition contiguous) and one identity matmul per (conv, tile).
  - per tile: PSUM->SBUF copies and per-partition dinv scaling on the
    Activation engine (all func=Copy, so no act-table reloads), fp32 GEMM
    with W on PE, add/bias/relu on DVE into a per-group output buffer,
    one output DMA per group ([128, T, 128] layout, untransposed on host).
  - dma_gather indices are int16, so each conv's edge stream is split by
    source-node half (< 32768 vs >= 32768): 4 streams total.
"""

import sys

if "/opt/trn_rl_repo" not in sys.path:
    sys.path.insert(0, "/opt/trn_rl_repo")

import numpy as np

P = 128
HALF = 32768  # int16 index limit for dma_gather
N_CORES = 8
SENT = 20000.0  # sentinel dst-slot for padded edge lanes (matches no r)


class Schedule:
    pass


def _assign_slots(cnt, n_tiles, T, n_cores, iters=30000, seed=0):
    """Assign tiles to (core, slot) minimizing sum_j sum_s ceil(max_8 / 128).

    cnt: [4, n_tiles] per-tile stream counts. Returns tile_core, tile_slot.
    """
    order = np.argsort(-cnt.sum(0), kind="stable")
    slot_tiles = [list(order[j * n_cores:(j + 1) * n_cores])
                  for j in range(T)]

    c = cnt.T  # [n_tiles, 4]

    def slot_cost(tiles):
        mx = c[tiles].max(axis=0)
        return int(np.sum(-(-mx // P)))

    costs = [slot_cost(st) for st in slot_tiles]
    rng = np.random.default_rng(seed)
    ra = rng.integers(0, T, iters)
    rb = rng.integers(0, T, iters)
    ri = rng.integers(0, n_cores, iters)
    rj = rng.integers(0, n_cores, iters)
    for a, b, i, j in zip(ra, rb, ri, rj):
        if a == b:
            continue
        sa, sb = slot_tiles[a], slot_tiles[b]
        sa[i], sb[j] = sb[j], sa[i]
        na, nb = slot_cost(sa), slot_cost(sb)
        if na + nb < costs[a] + costs[b]:
            costs[a], costs[b] = na, nb
        else:
            sa[i], sb[j] = sb[j], sa[i]

    tile_core = np.empty(n_tiles, dtype=np.int64)
    tile_slot = np.empty(n_tiles, dtype=np.int64)
    for j, st in enumerate(slot_tiles):
        for ci, tl in enumerate(st):
            tile_core[tl] = ci
            tile_slot[tl] = j
    return tile_core, tile_slot


def preprocess(x, edge_index, W_fwd, b_fwd, W_bwd, b_bwd,
               tbl_np=np.float16, m_np=np.float16,
               group_tiles=8, n_cores=N_CORES):
    N, D = x.shape
    assert D == P
    src = edge_index[0].astype(np.int64)
    dst = edge_index[1].astype(np.int64)

    deg_f = np.bincount(dst, minlength=N) + 1.0
    deg_b = np.bincount(src, minlength=N) + 1.0
    dinv_f = (1.0 / np.sqrt(deg_f)).astype(np.float32)
    dinv_b = (1.0 / np.sqrt(deg_b)).astype(np.float32)

    u_f = (dinv_f[:, None] * x).astype(tbl_np)
    u_b = (dinv_b[:, None] * x).astype(tbl_np)

    # ---- per-node stream counts (no self-loops; they go the direct path) ----
    f_lo = np.bincount(dst[src < HALF], minlength=N)
    f_hi = np.bincount(dst[src >= HALF], minlength=N)
    b_lo = np.bincount(src[dst < HALF], minlength=N)
    b_hi = np.bincount(src[dst >= HALF], minlength=N)
    total = f_lo + f_hi + b_lo + b_hi

    n_tiles = -(-N // P)
    n_tiles = -(-n_tiles // n_cores) * n_cores
    T = n_tiles // n_cores

    # node -> tile: snake round-robin by decreasing total weight
    order = np.argsort(-total, kind="stable")
    tile_of_rank = np.arange(N) % n_tiles
    sweep = np.arange(N) // n_tiles
    odd = (sweep % 2) == 1
    tile_of_rank[odd] = n_tiles - 1 - tile_of_rank[odd]
    node_tile = np.empty(N, dtype=np.int64)
    node_tile[order] = tile_of_rank
    node_pos = np.empty(N, dtype=np.int64)
    by_tile = np.argsort(node_tile, kind="stable")
    sorted_tiles = node_tile[by_tile]
    pos_seq = np.arange(N) - np.searchsorted(sorted_tiles, sorted_tiles)
    # alternate slot halves by arrival (heaviest first) so per-half edge
    # counts stay balanced for the 64-wide M split
    pos_seq = (pos_seq % 2) * 64 + pos_seq // 2
    node_pos[by_tile] = pos_seq
    assert node_pos.max() < P

    # per-tile (stream, slot-half) counts -> tile -> (core, slot) assignment
    # balanced over all 8 columns (4 streams x 2 halves of the 64-wide M)
    cnt2 = np.zeros((4, n_tiles, 2), dtype=np.int64)
    key_sets = [dst[src < HALF], dst[src >= HALF],
                src[dst < HALF], src[dst >= HALF]]
    for i, kk in enumerate(key_sets):
        np.add.at(cnt2[i], (node_tile[kk],
                            (node_pos[kk] >= 64).astype(np.int64)), 1)
    tile_core, tile_slot = _assign_slots(
        cnt2.transpose(0, 2, 1).reshape(8, n_tiles), n_tiles, T, n_cores)

    # ---- edge streams: (f,lo) (f,hi) (b,lo) (b,hi) ----
    # within each (stream, tile) section, slot-half 0 (pos<64) edges come
    # first so M matrices can be 64 columns wide (pos stored mod 64)
    def split(keys, gidx):
        lo = gidx < HALF
        res = []
        for mask, bse in ((lo, 0), (~lo, HALF)):
            k = keys[mask]
            g = gidx[mask] - bse
            t = node_tile[k]
            hh = (node_pos[k] >= 64).astype(np.int64)
            o = np.lexsort((g, hh, t))
            res.append((t[o], g[o], node_pos[k[o]]))
        return res

    streams = split(dst, src) + split(src, dst)

    # per-half chunk counts K2 (max across the 8 cores sharing each slot)
    K2 = np.zeros((4, T, 2), dtype=np.int64)
    for s in range(4):
        for h in range(2):
            per_slot = np.zeros(T, dtype=np.int64)
            np.maximum.at(per_slot, tile_slot, cnt2[s, :, h])
            K2[s, :, h] = -(-per_slot // P)
    K = K2.sum(-1)  # overrides the single-width chunk counts
    kmax = max(int(K.max()), 1)

    # ---- static schedule layout ----
    groups = [list(range(g, min(g + group_tiles, T)))
              for g in range(0, T, group_tiles)]

    pos_off = np.zeros((4, T), dtype=np.int64)
    c = 0
    for s in range(4):
        for t in range(T):
            pos_off[s, t] = c
            c += K[s, t]
    pos_cols = max(c, 1)

    idx_meta = []  # per group: list of (col_start, n_idx) per stream
    c = 0
    for g in groups:
        row = []
        for s in range(4):
            n_idx = int(K[s, list(g)].sum()) * P
            row.append((c, n_idx))
            c += n_idx // 16
        idx_meta.append(row)
    idx_cols = max(c, 1)

    # ---- per-core packing ----
    slot_node = np.full((n_cores, T * P), -1, dtype=np.int64)
    g_slot = tile_slot[node_tile] * P + node_pos
    for core in range(n_cores):
        m = tile_core[node_tile] == core
        slot_node[core, g_slot[m]] = np.arange(N)[m]

    bias_sum = (b_fwd + b_bwd).astype(np.float32)
    wf = np.ascontiguousarray(W_fwd.astype(m_np))
    wb = np.ascontiguousarray(W_bwd.astype(m_np))
    bias_mat = np.ascontiguousarray(
        np.broadcast_to(bias_sum[None, :], (P, P))).astype(np.float32)
    iota_mat = np.ascontiguousarray(
        np.broadcast_to(np.arange(64, dtype=np.float32)[None, None, :],
                        (P, kmax, 64))).astype(m_np)
    ident = np.eye(P, dtype=np.float32).astype(m_np)

    in_maps = []
    for core in range(n_cores):
        gidx_arr = np.zeros((4, T, 2 * kmax * P), dtype=np.int16)
        pos_arr = np.full((4, T, 2 * kmax * P), SENT, dtype=np.float32)
        for s in range(4):
            tt, gg, pp = streams[s]
            m = tile_core[tt] == core
            t_loc = tile_slot[tt[m]]
            g_loc = gg[m]
            p_loc = pp[m]
            o = np.argsort(t_loc, kind="stable")
            t_loc, g_loc, p_loc = t_loc[o], g_loc[o], p_loc[o]
            starts = np.searchsorted(t_loc, np.arange(T))
            ends = np.searchsorted(t_loc, np.arange(T), side="right")
            for t in range(T):
                s0, e0 = int(starts[t]), int(ends[t])
                gl, pl = g_loc[s0:e0], p_loc[s0:e0]
                b = int(np.count_nonzero(pl < 64))
                k0 = int(K2[s, t, 0])
                assert b <= k0 * P and (len(pl) - b) <= int(K2[s, t, 1]) * P
                gidx_arr[s, t, :b] = gl[:b]
                pos_arr[s, t, :b] = pl[:b]
                gidx_arr[s, t, k0 * P:k0 * P + len(pl) - b] = gl[b:]
                pos_arr[s, t, k0 * P:k0 * P + len(pl) - b] = pl[b:] - 64

        idx_t = np.zeros((16, idx_cols), dtype=np.int16)
        for gi, g in enumerate(groups):
            for s in range(4):
                col0, n_idx = idx_meta[gi][s]
                if n_idx == 0:
                    continue
                flat = np.concatenate(
                    [gidx_arr[s, t, :int(K[s, t]) * P] for t in g])
                assert flat.shape[0] == n_idx
                idx_t[:, col0:col0 + n_idx // 16] = flat.reshape(-1, 16).T

        pos_t = np.zeros((P, pos_cols), dtype=np.float32)
        for s in range(4):
            for t in range(T):
                kst = int(K[s, t])
                if kst:
                    pos_t[:, pos_off[s, t]:pos_off[s, t] + kst] = \
                        pos_arr[s, t, :kst * P].reshape(kst, P).T
        pos_t = pos_t.astype(m_np)

        dvf = np.zeros((P, T), dtype=np.float32)
        dvb = np.zeros((P, T), dtype=np.float32)
        sn = slot_node[core].reshape(T, P)
        valid = sn >= 0
        dvf.T[valid] = dinv_f[sn[valid]]
        dvb.T[valid] = dinv_b[sn[valid]]

        # slot-permuted self tables (zero rows for dummy slots), stored
        # transposed [128, T, 128] so one contiguous DMA covers a group
        ufp = np.zeros((T * P, P), dtype=tbl_np)
        ubp = np.zeros((T * P, P), dtype=tbl_np)
        snf = slot_node[core]
        vm = snf >= 0
        ufp[vm] = u_f[snf[vm]]
        ubp[vm] = u_b[snf[vm]]
        ufpT = np.ascontiguousarray(
            ufp.reshape(T, P, P).transpose(1, 0, 2))
        ubpT = np.ascontiguousarray(
            ubp.reshape(T, P, P).transpose(1, 0, 2))

        in_maps.append({
            "uf": u_f, "ub": u_b, "ufp": ufpT, "ubp": ubpT,
            "idx": np.tile(idx_t, (8, 1)), "pos": pos_t,
            "dinvf": np.ascontiguousarray(dvf),
            "dinvb": np.ascontiguousarray(dvb),
            "wf": wf, "wb": wb, "bias": bias_mat,
            "iota": iota_mat, "ident": ident,
        })

    sch = Schedule()
    sch.T, sch.K, sch.groups = T, K, groups
    sch.K2 = K2
    sch.idx_meta, sch.pos_off = idx_meta, pos_off
    sch.idx_cols, sch.pos_cols = idx_cols, pos_cols
    sch.in_maps = in_maps
    sch.slot_node = slot_node
    sch.kmax = kmax
    sch.N, sch.n_cores = N, n_cores
    sch.tbl_np, sch.m_np = tbl_np, m_np
    return sch


# ---------------------------------------------------------------------------
# device program
# ---------------------------------------------------------------------------

def build_program(sch, debug=False, dup=1):
    from contextlib import ExitStack
    import concourse.mybir as mybir
    import concourse.tile as tile
    from concourse import bacc

    tbl_dt = mybir.dt.from_np(np.dtype(sch.tbl_np))
    m_dt = mybir.dt.from_np(np.dtype(sch.m_np))
    f32 = mybir.dt.float32
    T, K, kmax = sch.T, sch.K, sch.kmax
    N = sch.N

    nc = bacc.Bacc("TRN2", target_bir_lowering=False, debug=debug,
                   num_devices=sch.n_cores, num_swdge_queues=4)

    uf = nc.dram_tensor("uf", [N, P], tbl_dt, kind="ExternalInput").ap()
    ub = nc.dram_tensor("ub", [N, P], tbl_dt, kind="ExternalInput").ap()
    ufp = nc.dram_tensor("ufp", [P, T, P], tbl_dt,
                         kind="ExternalInput").ap()
    ubp = nc.dram_tensor("ubp", [P, T, P], tbl_dt,
                         kind="ExternalInput").ap()
    idx_d = nc.dram_tensor("idx", [128, sch.idx_cols], mybir.dt.int16,
                           kind="ExternalInput").ap()
    pos_d = nc.dram_tensor("pos", [P, sch.pos_cols], m_dt,
                           kind="ExternalInput").ap()
    dinvf_d = nc.dram_tensor("dinvf", [P, T], f32, kind="ExternalInput").ap()
    dinvb_d = nc.dram_tensor("dinvb", [P, T], f32, kind="ExternalInput").ap()
    wf_d = nc.dram_tensor("wf", [P, P], m_dt, kind="ExternalInput").ap()
    wb_d = nc.dram_tensor("wb", [P, P], m_dt, kind="ExternalInput").ap()
    bias_d = nc.dram_tensor("bias", [P, P], f32, kind="ExternalInput").ap()
    iota_d = nc.dram_tensor("iota", [P, kmax, 64], m_dt,
                            kind="ExternalInput").ap()
    ident_d = nc.dram_tensor("ident", [P, P], m_dt,
                             kind="ExternalInput").ap()
    out_d = nc.dram_tensor("out", [P, T, P], f32, kind="ExternalOutput").ap()

    lim = min(HALF, N)
    tables = [uf[0:lim, :], uf[lim:N, :], ub[0:lim, :], ub[lim:N, :]]

    with tile.TileContext(nc) as tc, ExitStack() as ctx:
        const = ctx.enter_context(tc.tile_pool(name="const", bufs=1))

        def load_const(shape, dt, dram_ap, tag):
            t = const.tile(shape, dtype=dt, tag=tag)
            nc.sync.dma_start(out=t[:], in_=dram_ap)
            return t

        idx_sb = load_const([128, sch.idx_cols], mybir.dt.int16, idx_d,
                            "idx")
        pos_sb = load_const([P, sch.pos_cols], m_dt, pos_d, "pos")
        dinvf_sb = load_const([P, T], f32, dinvf_d, "dinvf")
        dinvb_sb = load_const([P, T], f32, dinvb_d, "dinvb")
        wf_sb = load_const([P, P], m_dt, wf_d, "wf")
        wb_sb = load_const([P, P], m_dt, wb_d, "wb")
        bias_sb = load_const([P, P], f32, bias_d, "bias")
        iota_sb = load_const([P, kmax, 64], m_dt, iota_d, "iota")
        ident_sb = load_const([P, P], m_dt, ident_d, "ident")

        gpools = [ctx.enter_context(tc.tile_pool(name=f"g{s}", bufs=2))
                  for s in range(4)]
        spool = ctx.enter_context(tc.tile_pool(name="selfp", bufs=2))
        mpool = ctx.enter_context(tc.tile_pool(name="m", bufs=4))
        aggp = ctx.enter_context(tc.tile_pool(name="aggp", bufs=2,
                                              space="PSUM"))
        outp = ctx.enter_context(tc.tile_pool(name="outp", bufs=2,
                                              space="PSUM"))
        sbp = ctx.enter_context(tc.tile_pool(name="sbp", bufs=4))
        obp = ctx.enter_context(tc.tile_pool(name="obp", bufs=2))

        qload = [0, 0, 0, 0]
        for _dup in range(dup):
            for gi, g in enumerate(sch.groups):
                gt0 = g[0]
                ng = len(g)
                gts = []
                for s in range(4):
                    col0, n_idx = sch.idx_meta[gi][s]
                    if n_idx == 0:
                        gts.append(None)
                        continue
                    n_chunks = n_idx // P
                    gt = gpools[s].tile([P, n_chunks, P], dtype=tbl_dt,
                                        tag=f"g{s}")
                    # ~3072 idxs per dma_gather (verified OK on HW)
                    for c0 in range(0, n_chunks, 24):
                        nsub = min(24, n_chunks - c0)
                        # queue aligned to the round-robin DMASW sem lane so
                        # each lane sees a single queue (sim invariant)
                        q = (qload[0] % 8) % 4
                        qload[0] += 1
                        nc.gpsimd.dma_gather(
                            out_ap=gt[:, c0:c0 + nsub, :],
                            in_ap=tables[s],
                            idxs_ap=idx_sb[:, col0 + c0 * 8:
                                           col0 + (c0 + nsub) * 8],
                            num_idxs=nsub * P,
                            num_idxs_reg=nsub * P,
                            elem_size=P,
                            queue_num=q,
                            single_packet=False,
                        )
                    gts.append(gt)

                selfF = spool.tile([P, ng, P], dtype=tbl_dt, tag="selfF")
                nc.sync.dma_start(out=selfF[:], in_=ufp[:, gt0:gt0 + ng, :])
                selfB = spool.tile([P, ng, P], dtype=tbl_dt, tag="selfB")
                nc.sync.dma_start(out=selfB[:], in_=ubp[:, gt0:gt0 + ng, :])
                ob = obp.tile([P, ng, P], dtype=f32, tag="ob")

                base = [0, 0, 0, 0]
                for ti, t in enumerate(g):
                    mts = []
                    for s in range(4):
                        kst = int(K[s, t])
                        if kst == 0:
                            mts.append(None)
                            continue
                        # one 64-wide is_equal covers both slot halves: the
                        # half-1 section's pos values are stored mod 64
                        mt = mpool.tile([P, kst, 64], dtype=m_dt, tag=f"m{s}")
                        po = int(sch.pos_off[s, t])
                        nc.vector.tensor_tensor(
                            out=mt[:],
                            in0=pos_sb[:, po:po + kst, None].to_broadcast(
                                [P, kst, 64]),
                            in1=iota_sb[:, 0:kst, :],
                            op=mybir.AluOpType.is_equal,
                        )
                        mts.append(mt)

                    def accum(psum, s_list, self_tile):
                        n_tot = sum(int(K[s, t]) for s in s_list) + 1
                        nc.tensor.matmul(out=psum[:],
                                         lhsT=self_tile[:, ti, :],
                                         rhs=ident_sb[:], start=True,
                                         stop=(n_tot == 1))
                        ci = 1
                        for s in s_list:
                            k0 = int(sch.K2[s, t, 0])
                            for c in range(int(K[s, t])):
                                ocols = (psum[:, 0:64] if c < k0
                                         else psum[:, 64:128])
                                nc.tensor.matmul(
                                    out=ocols,
                                    lhsT=gts[s][:, base[s] + c, :],
                                    rhs=mts[s][:, c, :],
                                    start=False,
                                    stop=(ci == n_tot - 1),
                                )
                                ci += 1

                    aggf = aggp.tile([P, P], dtype=f32, tag="aggf")
                    accum(aggf, (0, 1), selfF)
                    aggb = aggp.tile([P, P], dtype=f32, tag="aggb")
                    accum(aggb, (2, 3), selfB)

                    # PSUM->SBUF copies on Act (func=Copy); fp16 out so
                    # the W GEMM runs at 1 cycle/row on PE
                    af = sbp.tile([P, P], dtype=tbl_dt, tag="af")
                    nc.scalar.copy(out=af[:], in_=aggf[:])
                    ab = sbp.tile([P, P], dtype=tbl_dt, tag="ab")
                    nc.scalar.copy(out=ab[:], in_=aggb[:])

                    outf = outp.tile([P, P], dtype=f32, tag="outf")
                    nc.tensor.matmul(out=outf[:], lhsT=af[:], rhs=wf_sb[:],
                                     start=True, stop=True)
                    outb = outp.tile([P, P], dtype=f32, tag="outb")
                    nc.tensor.matmul(out=outb[:], lhsT=ab[:], rhs=wb_sb[:],
                                     start=True, stop=True)

                    # dinv scaling on Act (func=Copy w/ per-partition scale)
                    s1 = sbp.tile([P, P], dtype=f32, tag="s1")
                    nc.scalar.mul(out=s1[:], in_=outf[:],
                                  mul=dinvf_sb[:, t:t + 1])
                    s2 = sbp.tile([P, P], dtype=f32, tag="s2")
                    nc.scalar.mul(out=s2[:], in_=outb[:],
                                  mul=dinvb_sb[:, t:t + 1])
                    s3 = sbp.tile([P, P], dtype=f32, tag="s3")
                    nc.vector.tensor_tensor(out=s3[:], in0=s1[:], in1=s2[:],
                                            op=mybir.AluOpType.add)
                    s4 = sbp.tile([P, P], dtype=f32, tag="s4")
                    nc.vector.tensor_tensor(out=s4[:], in0=s3[:],
                                            in1=bias_sb[:],
                                            op=mybir.AluOpType.add)
                    nc.vector.tensor_scalar(
                        out=ob[:, ti, :], in0=s4[:], scalar1=0.0,
                        scalar2=None, op0=mybir.AluOpType.max)

                    for s in range(4):
                        base[s] += int(K[s, t])

                nc.sync.dma_start(out=out_d[:, gt0:gt0 + ng, :], in_=ob[:])

    nc.compile()
    return nc


# ---------------------------------------------------------------------------
# entry point
# ---------------------------------------------------------------------------

_CACHE = {}


def run_sch(sch, trace=False, **kw):
    from concourse.bass_utils import run_bass_kernel_spmd
    key = ("prog", sch.T, tuple(map(tuple, sch.K.tolist())), sch.idx_cols,
           sch.pos_cols, str(np.dtype(sch.tbl_np)), str(np.dtype(sch.m_np)))
    if key not in _CACHE:
        _CACHE.clear()
        _CACHE[key] = build_program(sch)
    nc = _CACHE[key]
    return run_bass_kernel_spmd(
        nc, sch.in_maps, core_ids=list(range(sch.n_cores)), trace=trace, **kw)


def assemble(sch, results):
    out = np.zeros((sch.N, P), dtype=np.float32)
    for core in range(sch.n_cores):
        o = results[core]["out"]  # [128, T, 128]; slot (t, p) at o[p, t]
        rows = o.transpose(1, 0, 2).reshape(-1, P)
        sn = sch.slot_node[core]
        m = sn >= 0
        out[sn[m]] = rows[m]
    return out


def kernel(x, edge_index, W_fwd, b_fwd, W_bwd, b_bwd):
    x = np.asarray(x, dtype=np.float32)
    edge_index = np.asarray(edge_index, dtype=np.int32)
    sch = preprocess(
        x, edge_index,
        np.asarray(W_fwd, np.float32), np.asarray(b_fwd, np.float32),
        np.asarray(W_bwd, np.float32), np.asarray(b_bwd, np.float32))
    res = run_sch(sch)
    return assemble(sch, res.results)



# revision 44
# speedup vs baseline: 1.7425x; 1.7425x over previous
"""Trainium2 Bass kernel for nn_DiffusionConv (two-direction GCN conv + relu).

out = relu(gcn(x, W_fwd; src->dst) + gcn(x, W_bwd; dst->src))

Algorithm (validated against the jax reference in numpy):
  gcn(x, W; edges) = D^-1/2 (A + I) D^-1/2 x W  with D = indegree+1.
  The weight GEMM commutes with aggregation, so the device aggregates scaled
  features u = dinv * x over edges (gather rows by source, reduce into
  destination tiles), applies W to each aggregated 128-row tile, scales by
  destination dinv, adds bias, relu.

Device mapping (one SPMD program on 8 cores):
  - nodes are permuted into tiles of 128 slots; tiles sharded across cores,
    processed in groups of 8 tiles per core.
  - per (group, stream), in-edges are chunked into groups of 128; batched
    dma_gather (3072 rows per call -- large calls amortize the ~1us SWDGE
    fixed cost, and keeping total DMA-instruction count low avoids a HW
    cliff; 4 SWDGE queues aligned to the 8 round-robin DMASW sem lanes)
    pulls source rows G [128e, 128f] from the fp16 table in DRAM.
  - M[e, r] = (dst_slot[e] == r) built on DVE with one is_equal against a
    precomputed iota matrix; PE matmul agg_T += G.T @ M accumulates in PSUM.
    M is 64 columns wide: each (stream, tile) edge section is ordered
    [slots 0-63 | slots 64-127] (pos stored mod 64, per-half chunk padding)
    and the matmuls target the matching PSUM column half -- halves both the
    DVE compare work and the PE matmul width.
  - self-loop terms use one direct DMA per group of the slot-permuted u
    tables (stored transposed [128, T, 128] in DRAM so the group slice is
    per-partition contiguous) and one identity matmul per (conv, tile).
  - per tile: PSUM->SBUF copies and per-partition dinv scaling on the
    Activation engine (all func=Copy, so no act-table reloads), fp32 GEMM
    with W on PE, add/bias/relu on DVE into a per-group output buffer,
    one output DMA per group ([128, T, 128] layout, untransposed on host).
  - dma_gather indices are int16, so each conv's edge stream is split by
    source-node half (< 32768 vs >= 32768): 4 streams total.
"""

import sys

if "/opt/trn_rl_repo" not in sys.path:
    sys.path.insert(0, "/opt/trn_rl_repo")

import numpy as np

P = 128
HALF = 32768  # int16 index limit for dma_gather
N_CORES = 8
SENT = 20000.0  # sentinel dst-slot for padded edge lanes (matches no r)


class Schedule:
    pass


def _assign_slots(cnt, n_tiles, T, n_cores, iters=30000, seed=0):
    """Assign tiles to (core, slot) minimizing sum_j sum_s ceil(max_8 / 128).

    cnt: [4, n_tiles] per-tile stream counts. Returns tile_core, tile_slot.
    """
    order = np.argsort(-cnt.sum(0), kind="stable")
    slot_tiles = [list(order[j * n_cores:(j + 1) * n_cores])
                  for j in range(T)]

    c = cnt.T  # [n_tiles, 4]

    def slot_cost(tiles):
        mx = c[tiles].max(axis=0)
        return int(np.sum(-(-mx // P)))

    costs = [slot_cost(st) for st in slot_tiles]
    rng = np.random.default_rng(seed)
    ra = rng.integers(0, T, iters)
    rb = rng.integers(0, T, iters)
    ri = rng.integers(0, n_cores, iters)
    rj = rng.integers(0, n_cores, iters)
    for a, b, i, j in zip(ra, rb, ri, rj):
        if a == b:
            continue
        sa, sb = slot_tiles[a], slot_tiles[b]
        sa[i], sb[j] = sb[j], sa[i]
        na, nb = slot_cost(sa), slot_cost(sb)
        if na + nb < costs[a] + costs[b]:
            costs[a], costs[b] = na, nb
        else:
            sa[i], sb[j] = sb[j], sa[i]

    tile_core = np.empty(n_tiles, dtype=np.int64)
    tile_slot = np.empty(n_tiles, dtype=np.int64)
    for j, st in enumerate(slot_tiles):
        for ci, tl in enumerate(st):
            tile_core[tl] = ci
            tile_slot[tl] = j
    return tile_core, tile_slot


def preprocess(x, edge_index, W_fwd, b_fwd, W_bwd, b_bwd,
               tbl_np=np.float16, m_np=np.float16,
               group_tiles=8, n_cores=N_CORES):
    N, D = x.shape
    assert D == P
    src = edge_index[0].astype(np.int64)
    dst = edge_index[1].astype(np.int64)

    deg_f = np.bincount(dst, minlength=N) + 1.0
    deg_b = np.bincount(src, minlength=N) + 1.0
    dinv_f = (1.0 / np.sqrt(deg_f)).astype(np.float32)
    dinv_b = (1.0 / np.sqrt(deg_b)).astype(np.float32)

    u_f = (dinv_f[:, None] * x).astype(tbl_np)
    u_b = (dinv_b[:, None] * x).astype(tbl_np)

    # ---- per-node stream counts (no self-loops; they go the direct path) ----
    f_lo = np.bincount(dst[src < HALF], minlength=N)
    f_hi = np.bincount(dst[src >= HALF], minlength=N)
    b_lo = np.bincount(src[dst < HALF], minlength=N)
    b_hi = np.bincount(src[dst >= HALF], minlength=N)
    total = f_lo + f_hi + b_lo + b_hi

    n_tiles = -(-N // P)
    n_tiles = -(-n_tiles // n_cores) * n_cores
    T = n_tiles // n_cores

    # node -> tile: snake round-robin by decreasing total weight
    order = np.argsort(-total, kind="stable")
    tile_of_rank = np.arange(N) % n_tiles
    sweep = np.arange(N) // n_tiles
    odd = (sweep % 2) == 1
    tile_of_rank[odd] = n_tiles - 1 - tile_of_rank[odd]
    node_tile = np.empty(N, dtype=np.int64)
    node_tile[order] = tile_of_rank
    node_pos = np.empty(N, dtype=np.int64)
    by_tile = np.argsort(node_tile, kind="stable")
    sorted_tiles = node_tile[by_tile]
    pos_seq = np.arange(N) - np.searchsorted(sorted_tiles, sorted_tiles)
    # alternate slot halves by arrival (heaviest first) so per-half edge
    # counts stay balanced for the 64-wide M split
    pos_seq = (pos_seq % 2) * 64 + pos_seq // 2
    node_pos[by_tile] = pos_seq
    assert node_pos.max() < P

    # per-tile (stream, slot-half) counts -> tile -> (core, slot) assignment
    # balanced over all 8 columns (4 streams x 2 halves of the 64-wide M)
    cnt2 = np.zeros((4, n_tiles, 2), dtype=np.int64)
    key_sets = [dst[src < HALF], dst[src >= HALF],
                src[dst < HALF], src[dst >= HALF]]
    for i, kk in enumerate(key_sets):
        np.add.at(cnt2[i], (node_tile[kk],
                            (node_pos[kk] >= 64).astype(np.int64)), 1)
    tile_core, tile_slot = _assign_slots(
        cnt2.transpose(0, 2, 1).reshape(8, n_tiles), n_tiles, T, n_cores)

    # ---- edge streams: (f,lo) (f,hi) (b,lo) (b,hi) ----
    # within each (stream, tile) section, slot-half 0 (pos<64) edges come
    # first so M matrices can be 64 columns wide (pos stored mod 64)
    def split(keys, gidx):
        lo = gidx < HALF
        res = []
        for mask, bse in ((lo, 0), (~lo, HALF)):
            k = keys[mask]
            g = gidx[mask] - bse
            t = node_tile[k]
            hh = (node_pos[k] >= 64).astype(np.int64)
            o = np.lexsort((g, hh, t))
            res.append((t[o], g[o], node_pos[k[o]]))
        return res

    streams = split(dst, src) + split(src, dst)

    # exact per-half section lengths (max across the 8 cores sharing each
    # slot, NOT rounded to chunks) -- sections share chunks at static
    # boundary offsets; boundary chunks get one matmul per touching section
    L2 = np.zeros((4, T, 2), dtype=np.int64)
    for s in range(4):
        for h in range(2):
            per_slot = np.zeros(T, dtype=np.int64)
            np.maximum.at(per_slot, tile_slot, cnt2[s, :, h])
            L2[s, :, h] = per_slot

    # ---- static schedule layout ----
    groups = [list(range(g, min(g + group_tiles, T)))
              for g in range(0, T, group_tiles)]

    sec_off = np.zeros((4, T, 2), dtype=np.int64)  # lane off within (g,s)
    spans = np.zeros((4, T, 2, 2), dtype=np.int64)  # [s,t,h] -> c_lo, span
    pos_off2 = np.zeros((4, T, 2), dtype=np.int64)
    idx_meta = []  # per group: list of (col_start, n_idx) per stream
    c = 0
    pc = 0
    for g in groups:
        row = []
        for s in range(4):
            off = 0
            for t in g:
                for h in range(2):
                    sec_off[s, t, h] = off
                    ln = int(L2[s, t, h])
                    if ln:
                        c_lo = off // P
                        span = -(-(off + ln) // P) - c_lo
                    else:
                        c_lo, span = 0, 0
                    spans[s, t, h] = (c_lo, span)
                    pos_off2[s, t, h] = pc
                    pc += span
                    off += ln
            n_idx = -(-off // P) * P  # pad stream tail to whole chunks
            row.append((c, n_idx))
            c += n_idx // 16
        idx_meta.append(row)
    idx_cols = max(c, 1)
    pos_cols = max(pc, 1)
    kmax = max(int(spans[..., 1].sum(-1).max()), 1)
    K = L2.sum(-1)  # informational (chunks estimate for logging)

    # ---- per-core packing ----
    slot_node = np.full((n_cores, T * P), -1, dtype=np.int64)
    g_slot = tile_slot[node_tile] * P + node_pos
    for core in range(n_cores):
        m = tile_core[node_tile] == core
        slot_node[core, g_slot[m]] = np.arange(N)[m]

    bias_sum = (b_fwd + b_bwd).astype(np.float32)
    wf = np.ascontiguousarray(W_fwd.astype(m_np))
    wb = np.ascontiguousarray(W_bwd.astype(m_np))
    bias_mat = np.ascontiguousarray(
        np.broadcast_to(bias_sum[None, :], (P, P))).astype(np.float32)
    iota_mat = np.ascontiguousarray(
        np.broadcast_to(np.arange(64, dtype=np.float32)[None, None, :],
                        (P, kmax, 64))).astype(m_np)
    ident = np.eye(P, dtype=np.float32).astype(m_np)

    in_maps = []
    for core in range(n_cores):
        idx_t = np.zeros((16, idx_cols), dtype=np.int16)
        pos_t = np.full((P, pos_cols), SENT, dtype=np.float32)
        for s in range(4):
            tt, gg, pp = streams[s]
            m = tile_core[tt] == core
            t_loc = tile_slot[tt[m]]
            g_loc = gg[m]
            p_loc = pp[m]
            o = np.argsort(t_loc, kind="stable")
            t_loc, g_loc, p_loc = t_loc[o], g_loc[o], p_loc[o]
            starts = np.searchsorted(t_loc, np.arange(T))
            ends = np.searchsorted(t_loc, np.arange(T), side="right")
            for gi, g in enumerate(groups):
                col0, n_idx = idx_meta[gi][s]
                if n_idx == 0:
                    continue
                flat = np.zeros(n_idx, dtype=np.int16)
                for t in g:
                    s0, e0 = int(starts[t]), int(ends[t])
                    gl, pl = g_loc[s0:e0], p_loc[s0:e0]
                    b = int(np.count_nonzero(pl < 64))
                    for h, (gv, pv) in enumerate(
                            ((gl[:b], pl[:b]), (gl[b:], pl[b:] - 64))):
                        cntc = len(gv)
                        assert cntc <= int(L2[s, t, h])
                        so = int(sec_off[s, t, h])
                        flat[so:so + cntc] = gv
                        c_lo, span = (int(spans[s, t, h, 0]),
                                      int(spans[s, t, h, 1]))
                        if span == 0:
                            continue
                        w = np.full(span * P, SENT, dtype=np.float32)
                        rel = so - c_lo * P
                        w[rel:rel + cntc] = pv
                        po = int(pos_off2[s, t, h])
                        pos_t[:, po:po + span] = w.reshape(span, P).T
                idx_t[:, col0:col0 + n_idx // 16] = flat.reshape(-1, 16).T
        pos_t = pos_t.astype(m_np)

        dvf = np.zeros((P, T), dtype=np.float32)
        dvb = np.zeros((P, T), dtype=np.float32)
        sn = slot_node[core].reshape(T, P)
        valid = sn >= 0
        dvf.T[valid] = dinv_f[sn[valid]]
        dvb.T[valid] = dinv_b[sn[valid]]

        # slot-permuted self tables (zero rows for dummy slots), stored
        # transposed [128, T, 128] so one contiguous DMA covers a group
        ufp = np.zeros((T * P, P), dtype=tbl_np)
        ubp = np.zeros((T * P, P), dtype=tbl_np)
        snf = slot_node[core]
        vm = snf >= 0
        ufp[vm] = u_f[snf[vm]]
        ubp[vm] = u_b[snf[vm]]
        ufpT = np.ascontiguousarray(
            ufp.reshape(T, P, P).transpose(1, 0, 2))
        ubpT = np.ascontiguousarray(
            ubp.reshape(T, P, P).transpose(1, 0, 2))

        in_maps.append({
            "uf": u_f, "ub": u_b, "ufp": ufpT, "ubp": ubpT,
            "idx": np.tile(idx_t, (8, 1)), "pos": pos_t,
            "dinvf": np.ascontiguousarray(dvf),
            "dinvb": np.ascontiguousarray(dvb),
            "wf": wf, "wb": wb, "bias": bias_mat,
            "iota": iota_mat, "ident": ident,
        })

    sch = Schedule()
    sch.T, sch.K, sch.groups = T, K, groups
    sch.spans, sch.pos_off2 = spans, pos_off2
    sch.idx_meta = idx_meta
    sch.idx_cols, sch.pos_cols = idx_cols, pos_cols
    sch.in_maps = in_maps
    sch.slot_node = slot_node
    sch.kmax = kmax
    sch.N, sch.n_cores = N, n_cores
    sch.tbl_np, sch.m_np = tbl_np, m_np
    return sch


# ---------------------------------------------------------------------------
# device program
# ---------------------------------------------------------------------------

def build_program(sch, debug=False, dup=1):
    from contextlib import ExitStack
    import concourse.mybir as mybir
    import concourse.tile as tile
    from concourse import bacc

    tbl_dt = mybir.dt.from_np(np.dtype(sch.tbl_np))
    m_dt = mybir.dt.from_np(np.dtype(sch.m_np))
    f32 = mybir.dt.float32
    T, K, kmax = sch.T, sch.K, sch.kmax
    N = sch.N

    nc = bacc.Bacc("TRN2", target_bir_lowering=False, debug=debug,
                   num_devices=sch.n_cores, num_swdge_queues=4)

    uf = nc.dram_tensor("uf", [N, P], tbl_dt, kind="ExternalInput").ap()
    ub = nc.dram_tensor("ub", [N, P], tbl_dt, kind="ExternalInput").ap()
    ufp = nc.dram_tensor("ufp", [P, T, P], tbl_dt,
                         kind="ExternalInput").ap()
    ubp = nc.dram_tensor("ubp", [P, T, P], tbl_dt,
                         kind="ExternalInput").ap()
    idx_d = nc.dram_tensor("idx", [128, sch.idx_cols], mybir.dt.int16,
                           kind="ExternalInput").ap()
    pos_d = nc.dram_tensor("pos", [P, sch.pos_cols], m_dt,
                           kind="ExternalInput").ap()
    dinvf_d = nc.dram_tensor("dinvf", [P, T], f32, kind="ExternalInput").ap()
    dinvb_d = nc.dram_tensor("dinvb", [P, T], f32, kind="ExternalInput").ap()
    wf_d = nc.dram_tensor("wf", [P, P], m_dt, kind="ExternalInput").ap()
    wb_d = nc.dram_tensor("wb", [P, P], m_dt, kind="ExternalInput").ap()
    bias_d = nc.dram_tensor("bias", [P, P], f32, kind="ExternalInput").ap()
    iota_d = nc.dram_tensor("iota", [P, kmax, 64], m_dt,
                            kind="ExternalInput").ap()
    ident_d = nc.dram_tensor("ident", [P, P], m_dt,
                             kind="ExternalInput").ap()
    out_d = nc.dram_tensor("out", [P, T, P], f32, kind="ExternalOutput").ap()

    lim = min(HALF, N)
    tables = [uf[0:lim, :], uf[lim:N, :], ub[0:lim, :], ub[lim:N, :]]

    with tile.TileContext(nc) as tc, ExitStack() as ctx:
        const = ctx.enter_context(tc.tile_pool(name="const", bufs=1))

        def load_const(shape, dt, dram_ap, tag):
            t = const.tile(shape, dtype=dt, tag=tag)
            nc.sync.dma_start(out=t[:], in_=dram_ap)
            return t

        idx_sb = load_const([128, sch.idx_cols], mybir.dt.int16, idx_d,
                            "idx")
        pos_sb = load_const([P, sch.pos_cols], m_dt, pos_d, "pos")
        dinvf_sb = load_const([P, T], f32, dinvf_d, "dinvf")
        dinvb_sb = load_const([P, T], f32, dinvb_d, "dinvb")
        wf_sb = load_const([P, P], m_dt, wf_d, "wf")
        wb_sb = load_const([P, P], m_dt, wb_d, "wb")
        bias_sb = load_const([P, P], f32, bias_d, "bias")
        iota_sb = load_const([P, kmax, 64], m_dt, iota_d, "iota")
        ident_sb = load_const([P, P], m_dt, ident_d, "ident")

        gpools = [ctx.enter_context(tc.tile_pool(name=f"g{s}", bufs=2))
                  for s in range(4)]
        spool = ctx.enter_context(tc.tile_pool(name="selfp", bufs=2))
        mpool = ctx.enter_context(tc.tile_pool(name="m", bufs=4))
        aggp = ctx.enter_context(tc.tile_pool(name="aggp", bufs=2,
                                              space="PSUM"))
        outp = ctx.enter_context(tc.tile_pool(name="outp", bufs=2,
                                              space="PSUM"))
        sbp = ctx.enter_context(tc.tile_pool(name="sbp", bufs=4))
        obp = ctx.enter_context(tc.tile_pool(name="obp", bufs=2))

        qload = [0, 0, 0, 0]
        for _dup in range(dup):
            for gi, g in enumerate(sch.groups):
                gt0 = g[0]
                ng = len(g)
                gts = []
                for s in range(4):
                    col0, n_idx = sch.idx_meta[gi][s]
                    if n_idx == 0:
                        gts.append(None)
                        continue
                    n_chunks = n_idx // P
                    gt = gpools[s].tile([P, n_chunks, P], dtype=tbl_dt,
                                        tag=f"g{s}")
                    # ~3072 idxs per dma_gather (verified OK on HW)
                    for c0 in range(0, n_chunks, 24):
                        nsub = min(24, n_chunks - c0)
                        # queue aligned to the round-robin DMASW sem lane so
                        # each lane sees a single queue (sim invariant)
                        q = (qload[0] % 8) % 4
                        qload[0] += 1
                        nc.gpsimd.dma_gather(
                            out_ap=gt[:, c0:c0 + nsub, :],
                            in_ap=tables[s],
                            idxs_ap=idx_sb[:, col0 + c0 * 8:
                                           col0 + (c0 + nsub) * 8],
                            num_idxs=nsub * P,
                            num_idxs_reg=nsub * P,
                            elem_size=P,
                            queue_num=q,
                            single_packet=False,
                        )
                    gts.append(gt)

                selfF = spool.tile([P, ng, P], dtype=tbl_dt, tag="selfF")
                nc.sync.dma_start(out=selfF[:], in_=ufp[:, gt0:gt0 + ng, :])
                selfB = spool.tile([P, ng, P], dtype=tbl_dt, tag="selfB")
                nc.sync.dma_start(out=selfB[:], in_=ubp[:, gt0:gt0 + ng, :])
                ob = obp.tile([P, ng, P], dtype=f32, tag="ob")

                for ti, t in enumerate(g):
                    # the two half-sections' pos columns are adjacent, so one
                    # 64-wide is_equal per (stream, tile) builds both halves'
                    # M; foreign lanes in shared boundary chunks read SENT
                    mts = {}
                    for s in range(4):
                        sp0 = int(sch.spans[s, t, 0, 1])
                        sp1 = int(sch.spans[s, t, 1, 1])
                        if sp0 + sp1 == 0:
                            continue
                        mt = mpool.tile([P, sp0 + sp1, 64], dtype=m_dt,
                                        tag=f"m{s}")
                        po = int(sch.pos_off2[s, t, 0])
                        nc.vector.tensor_tensor(
                            out=mt[:],
                            in0=pos_sb[:, po:po + sp0 + sp1,
                                       None].to_broadcast([P, sp0 + sp1, 64]),
                            in1=iota_sb[:, 0:sp0 + sp1, :],
                            op=mybir.AluOpType.is_equal,
                        )
                        mts[s] = mt

                    def accum(psum, s_list, self_tile):
                        parts = []
                        for s in s_list:
                            sp0 = int(sch.spans[s, t, 0, 1])
                            for h in range(2):
                                c_lo = int(sch.spans[s, t, h, 0])
                                span = int(sch.spans[s, t, h, 1])
                                if span:
                                    parts.append(
                                        (s, h, c_lo, span, sp0 * h))
                        n_tot = 1 + sum(pp[3] for pp in parts)
                        nc.tensor.matmul(out=psum[:],
                                         lhsT=self_tile[:, ti, :],
                                         rhs=ident_sb[:], start=True,
                                         stop=(n_tot == 1))
                        ci = 1
                        for s, h, c_lo, span, moff in parts:
                            ocols = psum[:, 0:64] if h == 0 else psum[:, 64:128]
                            for c in range(span):
                                nc.tensor.matmul(
                                    out=ocols,
                                    lhsT=gts[s][:, c_lo + c, :],
                                    rhs=mts[s][:, moff + c, :],
                                    start=False,
                                    stop=(ci == n_tot - 1),
                                )
                                ci += 1

                    aggf = aggp.tile([P, P], dtype=f32, tag="aggf")
                    accum(aggf, (0, 1), selfF)
                    aggb = aggp.tile([P, P], dtype=f32, tag="aggb")
                    accum(aggb, (2, 3), selfB)

                    # PSUM->SBUF copies on Act (func=Copy); fp16 out so
                    # the W GEMM runs at 1 cycle/row on PE
                    af = sbp.tile([P, P], dtype=tbl_dt, tag="af")
                    nc.scalar.copy(out=af[:], in_=aggf[:])
                    ab = sbp.tile([P, P], dtype=tbl_dt, tag="ab")
                    nc.scalar.copy(out=ab[:], in_=aggb[:])

                    outf = outp.tile([P, P], dtype=f32, tag="outf")
                    nc.tensor.matmul(out=outf[:], lhsT=af[:], rhs=wf_sb[:],
                                     start=True, stop=True)
                    outb = outp.tile([P, P], dtype=f32, tag="outb")
                    nc.tensor.matmul(out=outb[:], lhsT=ab[:], rhs=wb_sb[:],
                                     start=True, stop=True)

                    # dinv scaling on Act (func=Copy w/ per-partition scale)
                    s1 = sbp.tile([P, P], dtype=f32, tag="s1")
                    nc.scalar.mul(out=s1[:], in_=outf[:],
                                  mul=dinvf_sb[:, t:t + 1])
                    s2 = sbp.tile([P, P], dtype=f32, tag="s2")
                    nc.scalar.mul(out=s2[:], in_=outb[:],
                                  mul=dinvb_sb[:, t:t + 1])
                    s3 = sbp.tile([P, P], dtype=f32, tag="s3")
                    nc.vector.tensor_tensor(out=s3[:], in0=s1[:], in1=s2[:],
                                            op=mybir.AluOpType.add)
                    s4 = sbp.tile([P, P], dtype=f32, tag="s4")
                    nc.vector.tensor_tensor(out=s4[:], in0=s3[:],
                                            in1=bias_sb[:],
                                            op=mybir.AluOpType.add)
                    nc.vector.tensor_scalar(
                        out=ob[:, ti, :], in0=s4[:], scalar1=0.0,
                        scalar2=None, op0=mybir.AluOpType.max)

                nc.sync.dma_start(out=out_d[:, gt0:gt0 + ng, :], in_=ob[:])

    nc.compile()
    return nc


# ---------------------------------------------------------------------------
# entry point
# ---------------------------------------------------------------------------

_CACHE = {}


def run_sch(sch, trace=False, **kw):
    from concourse.bass_utils import run_bass_kernel_spmd
    key = ("prog", sch.T, sch.spans.tobytes(), sch.idx_cols,
           sch.pos_cols, str(np.dtype(sch.tbl_np)), str(np.dtype(sch.m_np)))
    if key not in _CACHE:
        _CACHE.clear()
        _CACHE[key] = build_program(sch)
    nc = _CACHE[key]
    return run_bass_kernel_spmd(
        nc, sch.in_maps, core_ids=list(range(sch.n_cores)), trace=trace, **kw)


def assemble(sch, results):
    out = np.zeros((sch.N, P), dtype=np.float32)
    for core in range(sch.n_cores):
        o = results[core]["out"]  # [128, T, 128]; slot (t, p) at o[p, t]
        rows = o.transpose(1, 0, 2).reshape(-1, P)
        sn = sch.slot_node[core]
        m = sn >= 0
        out[sn[m]] = rows[m]
    return out


def kernel(x, edge_index, W_fwd, b_fwd, W_bwd, b_bwd):
    x = np.asarray(x, dtype=np.float32)
    edge_index = np.asarray(edge_index, dtype=np.int32)
    sch = preprocess(
        x, edge_index,
        np.asarray(W_fwd, np.float32), np.asarray(b_fwd, np.float32),
        np.asarray(W_bwd, np.float32), np.asarray(b_bwd, np.float32))
    res = run_sch(sch)
    return assemble(sch, res.results)



# revision 45
# speedup vs baseline: 1.7431x; 1.0003x over previous
"""Trainium2 Bass kernel for nn_DiffusionConv (two-direction GCN conv + relu).

out = relu(gcn(x, W_fwd; src->dst) + gcn(x, W_bwd; dst->src))

Algorithm (validated against the jax reference in numpy):
  gcn(x, W; edges) = D^-1/2 (A + I) D^-1/2 x W  with D = indegree+1.
  The weight GEMM commutes with aggregation, so the device aggregates scaled
  features u = dinv * x over edges (gather rows by source, reduce into
  destination tiles), applies W to each aggregated 128-row tile, scales by
  destination dinv, adds bias, relu.

Device mapping (one SPMD program on 8 cores):
  - nodes are permuted into tiles of 128 slots; tiles sharded across cores,
    processed in groups of 8 tiles per core.
  - per (group, stream), in-edges are chunked into groups of 128; batched
    dma_gather (3072 rows per call -- large calls amortize the ~1us SWDGE
    fixed cost, and keeping total DMA-instruction count low avoids a HW
    cliff; 4 SWDGE queues aligned to the 8 round-robin DMASW sem lanes)
    pulls source rows G [128e, 128f] from the fp16 table in DRAM.
  - M[e, r] = (dst_slot[e] == r) built on DVE with one is_equal against a
    precomputed iota matrix; PE matmul agg_T += G.T @ M accumulates in PSUM.
    M is 64 columns wide: each (stream, tile) edge section is ordered
    [slots 0-63 | slots 64-127] (pos stored mod 64) and the matmuls target
    the matching PSUM column half -- halves both the DVE compare work and
    the PE matmul width. Sections are packed at their exact cross-core-max
    lengths (no rounding to 128): neighboring sections share boundary
    chunks, which get one matmul per touching section with foreign lanes
    SENT-masked, cutting gather descriptors ~16%.
  - self-loop terms use one direct DMA per group of the slot-permuted u
    tables (stored transposed [128, T, 128] in DRAM so the group slice is
    per-partition contiguous) and one identity matmul per (conv, tile).
  - per tile: PSUM->SBUF copies and per-partition dinv scaling on the
    Activation engine (all func=Copy, so no act-table reloads), fp32 GEMM
    with W on PE, add/bias/relu on DVE into a per-group output buffer,
    one output DMA per group ([128, T, 128] layout, untransposed on host).
  - dma_gather indices are int16, so each conv's edge stream is split by
    source-node half (< 32768 vs >= 32768): 4 streams total.
"""

import sys

if "/opt/trn_rl_repo" not in sys.path:
    sys.path.insert(0, "/opt/trn_rl_repo")

import numpy as np

P = 128
HALF = 32768  # int16 index limit for dma_gather
N_CORES = 8
SENT = 20000.0  # sentinel dst-slot for padded edge lanes (matches no r)


class Schedule:
    pass


def _assign_slots(cnt, n_tiles, T, n_cores, iters=30000, seed=0):
    """Assign tiles to (core, slot) minimizing sum_j sum_s ceil(max_8 / 128).

    cnt: [4, n_tiles] per-tile stream counts. Returns tile_core, tile_slot.
    """
    order = np.argsort(-cnt.sum(0), kind="stable")
    slot_tiles = [list(order[j * n_cores:(j + 1) * n_cores])
                  for j in range(T)]

    c = cnt.T  # [n_tiles, 4]

    def slot_cost(tiles):
        mx = c[tiles].max(axis=0)
        return int(np.sum(-(-mx // P)))

    costs = [slot_cost(st) for st in slot_tiles]
    rng = np.random.default_rng(seed)
    ra = rng.integers(0, T, iters)
    rb = rng.integers(0, T, iters)
    ri = rng.integers(0, n_cores, iters)
    rj = rng.integers(0, n_cores, iters)
    for a, b, i, j in zip(ra, rb, ri, rj):
        if a == b:
            continue
        sa, sb = slot_tiles[a], slot_tiles[b]
        sa[i], sb[j] = sb[j], sa[i]
        na, nb = slot_cost(sa), slot_cost(sb)
        if na + nb < costs[a] + costs[b]:
            costs[a], costs[b] = na, nb
        else:
            sa[i], sb[j] = sb[j], sa[i]

    tile_core = np.empty(n_tiles, dtype=np.int64)
    tile_slot = np.empty(n_tiles, dtype=np.int64)
    for j, st in enumerate(slot_tiles):
        for ci, tl in enumerate(st):
            tile_core[tl] = ci
            tile_slot[tl] = j
    return tile_core, tile_slot


def preprocess(x, edge_index, W_fwd, b_fwd, W_bwd, b_bwd,
               tbl_np=np.float16, m_np=np.float16,
               group_tiles=8, n_cores=N_CORES):
    N, D = x.shape
    assert D == P
    src = edge_index[0].astype(np.int64)
    dst = edge_index[1].astype(np.int64)

    deg_f = np.bincount(dst, minlength=N) + 1.0
    deg_b = np.bincount(src, minlength=N) + 1.0
    dinv_f = (1.0 / np.sqrt(deg_f)).astype(np.float32)
    dinv_b = (1.0 / np.sqrt(deg_b)).astype(np.float32)

    u_f = (dinv_f[:, None] * x).astype(tbl_np)
    u_b = (dinv_b[:, None] * x).astype(tbl_np)

    # ---- per-node stream counts (no self-loops; they go the direct path) ----
    f_lo = np.bincount(dst[src < HALF], minlength=N)
    f_hi = np.bincount(dst[src >= HALF], minlength=N)
    b_lo = np.bincount(src[dst < HALF], minlength=N)
    b_hi = np.bincount(src[dst >= HALF], minlength=N)
    total = f_lo + f_hi + b_lo + b_hi

    n_tiles = -(-N // P)
    n_tiles = -(-n_tiles // n_cores) * n_cores
    T = n_tiles // n_cores

    # node -> tile: snake round-robin by decreasing total weight
    order = np.argsort(-total, kind="stable")
    tile_of_rank = np.arange(N) % n_tiles
    sweep = np.arange(N) // n_tiles
    odd = (sweep % 2) == 1
    tile_of_rank[odd] = n_tiles - 1 - tile_of_rank[odd]
    node_tile = np.empty(N, dtype=np.int64)
    node_tile[order] = tile_of_rank
    node_pos = np.empty(N, dtype=np.int64)
    by_tile = np.argsort(node_tile, kind="stable")
    sorted_tiles = node_tile[by_tile]
    pos_seq = np.arange(N) - np.searchsorted(sorted_tiles, sorted_tiles)
    # alternate slot halves by arrival (heaviest first) so per-half edge
    # counts stay balanced for the 64-wide M split
    pos_seq = (pos_seq % 2) * 64 + pos_seq // 2
    node_pos[by_tile] = pos_seq
    assert node_pos.max() < P

    # per-tile (stream, slot-half) counts -> tile -> (core, slot) assignment
    # balanced over all 8 columns (4 streams x 2 halves of the 64-wide M)
    cnt2 = np.zeros((4, n_tiles, 2), dtype=np.int64)
    key_sets = [dst[src < HALF], dst[src >= HALF],
                src[dst < HALF], src[dst >= HALF]]
    for i, kk in enumerate(key_sets):
        np.add.at(cnt2[i], (node_tile[kk],
                            (node_pos[kk] >= 64).astype(np.int64)), 1)
    tile_core, tile_slot = _assign_slots(
        cnt2.transpose(0, 2, 1).reshape(8, n_tiles), n_tiles, T, n_cores)

    # ---- edge streams: (f,lo) (f,hi) (b,lo) (b,hi) ----
    # within each (stream, tile) section, slot-half 0 (pos<64) edges come
    # first so M matrices can be 64 columns wide (pos stored mod 64)
    def split(keys, gidx):
        lo = gidx < HALF
        res = []
        for mask, bse in ((lo, 0), (~lo, HALF)):
            k = keys[mask]
            g = gidx[mask] - bse
            t = node_tile[k]
            hh = (node_pos[k] >= 64).astype(np.int64)
            o = np.lexsort((g, hh, t))
            res.append((t[o], g[o], node_pos[k[o]]))
        return res

    streams = split(dst, src) + split(src, dst)

    # exact per-half section lengths (max across the 8 cores sharing each
    # slot, NOT rounded to chunks) -- sections share chunks at static
    # boundary offsets; boundary chunks get one matmul per touching section
    L2 = np.zeros((4, T, 2), dtype=np.int64)
    for s in range(4):
        for h in range(2):
            per_slot = np.zeros(T, dtype=np.int64)
            np.maximum.at(per_slot, tile_slot, cnt2[s, :, h])
            L2[s, :, h] = per_slot

    # ---- static schedule layout ----
    groups = [list(range(g, min(g + group_tiles, T)))
              for g in range(0, T, group_tiles)]

    sec_off = np.zeros((4, T, 2), dtype=np.int64)  # lane off within (g,s)
    spans = np.zeros((4, T, 2, 2), dtype=np.int64)  # [s,t,h] -> c_lo, span
    pos_off2 = np.zeros((4, T, 2), dtype=np.int64)
    idx_meta = []  # per group: list of (col_start, n_idx) per stream
    c = 0
    pc = 0
    for g in groups:
        row = []
        for s in range(4):
            off = 0
            for t in g:
                for h in range(2):
                    sec_off[s, t, h] = off
                    ln = int(L2[s, t, h])
                    if ln:
                        c_lo = off // P
                        span = -(-(off + ln) // P) - c_lo
                    else:
                        c_lo, span = 0, 0
                    spans[s, t, h] = (c_lo, span)
                    pos_off2[s, t, h] = pc
                    pc += span
                    off += ln
            n_idx = -(-off // P) * P  # pad stream tail to whole chunks
            row.append((c, n_idx))
            c += n_idx // 16
        idx_meta.append(row)
    idx_cols = max(c, 1)
    pos_cols = max(pc, 1)
    kmax = max(int(spans[..., 1].sum(-1).max()), 1)
    K = L2.sum(-1)  # informational (chunks estimate for logging)

    # ---- per-core packing ----
    slot_node = np.full((n_cores, T * P), -1, dtype=np.int64)
    g_slot = tile_slot[node_tile] * P + node_pos
    for core in range(n_cores):
        m = tile_core[node_tile] == core
        slot_node[core, g_slot[m]] = np.arange(N)[m]

    bias_sum = (b_fwd + b_bwd).astype(np.float32)
    wf = np.ascontiguousarray(W_fwd.astype(m_np))
    wb = np.ascontiguousarray(W_bwd.astype(m_np))
    bias_mat = np.ascontiguousarray(
        np.broadcast_to(bias_sum[None, :], (P, P))).astype(np.float32)
    iota_mat = np.ascontiguousarray(
        np.broadcast_to(np.arange(64, dtype=np.float32)[None, None, :],
                        (P, kmax, 64))).astype(m_np)
    ident = np.eye(P, dtype=np.float32).astype(m_np)

    in_maps = []
    for core in range(n_cores):
        idx_t = np.zeros((16, idx_cols), dtype=np.int16)
        pos_t = np.full((P, pos_cols), SENT, dtype=np.float32)
        for s in range(4):
            tt, gg, pp = streams[s]
            m = tile_core[tt] == core
            t_loc = tile_slot[tt[m]]
            g_loc = gg[m]
            p_loc = pp[m]
            o = np.argsort(t_loc, kind="stable")
            t_loc, g_loc, p_loc = t_loc[o], g_loc[o], p_loc[o]
            starts = np.searchsorted(t_loc, np.arange(T))
            ends = np.searchsorted(t_loc, np.arange(T), side="right")
            for gi, g in enumerate(groups):
                col0, n_idx = idx_meta[gi][s]
                if n_idx == 0:
                    continue
                flat = np.zeros(n_idx, dtype=np.int16)
                for t in g:
                    s0, e0 = int(starts[t]), int(ends[t])
                    gl, pl = g_loc[s0:e0], p_loc[s0:e0]
                    b = int(np.count_nonzero(pl < 64))
                    for h, (gv, pv) in enumerate(
                            ((gl[:b], pl[:b]), (gl[b:], pl[b:] - 64))):
                        cntc = len(gv)
                        assert cntc <= int(L2[s, t, h])
                        so = int(sec_off[s, t, h])
                        flat[so:so + cntc] = gv
                        c_lo, span = (int(spans[s, t, h, 0]),
                                      int(spans[s, t, h, 1]))
                        if span == 0:
                            continue
                        w = np.full(span * P, SENT, dtype=np.float32)
                        rel = so - c_lo * P
                        w[rel:rel + cntc] = pv
                        po = int(pos_off2[s, t, h])
                        pos_t[:, po:po + span] = w.reshape(span, P).T
                idx_t[:, col0:col0 + n_idx // 16] = flat.reshape(-1, 16).T
        pos_t = pos_t.astype(m_np)

        dvf = np.zeros((P, T), dtype=np.float32)
        dvb = np.zeros((P, T), dtype=np.float32)
        sn = slot_node[core].reshape(T, P)
        valid = sn >= 0
        dvf.T[valid] = dinv_f[sn[valid]]
        dvb.T[valid] = dinv_b[sn[valid]]

        # slot-permuted self tables (zero rows for dummy slots), stored
        # transposed [128, T, 128] so one contiguous DMA covers a group
        ufp = np.zeros((T * P, P), dtype=tbl_np)
        ubp = np.zeros((T * P, P), dtype=tbl_np)
        snf = slot_node[core]
        vm = snf >= 0
        ufp[vm] = u_f[snf[vm]]
        ubp[vm] = u_b[snf[vm]]
        ufpT = np.ascontiguousarray(
            ufp.reshape(T, P, P).transpose(1, 0, 2))
        ubpT = np.ascontiguousarray(
            ubp.reshape(T, P, P).transpose(1, 0, 2))

        in_maps.append({
            "uf": u_f, "ub": u_b, "ufp": ufpT, "ubp": ubpT,
            "idx": np.tile(idx_t, (8, 1)), "pos": pos_t,
            "dinvf": np.ascontiguousarray(dvf),
            "dinvb": np.ascontiguousarray(dvb),
            "wf": wf, "wb": wb, "bias": bias_mat,
            "iota": iota_mat, "ident": ident,
        })

    sch = Schedule()
    sch.T, sch.K, sch.groups = T, K, groups
    sch.spans, sch.pos_off2 = spans, pos_off2
    sch.idx_meta = idx_meta
    sch.idx_cols, sch.pos_cols = idx_cols, pos_cols
    sch.in_maps = in_maps
    sch.slot_node = slot_node
    sch.kmax = kmax
    sch.N, sch.n_cores = N, n_cores
    sch.tbl_np, sch.m_np = tbl_np, m_np
    return sch


# ---------------------------------------------------------------------------
# device program
# ---------------------------------------------------------------------------

def build_program(sch, debug=False, dup=1):
    from contextlib import ExitStack
    import concourse.mybir as mybir
    import concourse.tile as tile
    from concourse import bacc

    tbl_dt = mybir.dt.from_np(np.dtype(sch.tbl_np))
    m_dt = mybir.dt.from_np(np.dtype(sch.m_np))
    f32 = mybir.dt.float32
    T, K, kmax = sch.T, sch.K, sch.kmax
    N = sch.N

    nc = bacc.Bacc("TRN2", target_bir_lowering=False, debug=debug,
                   num_devices=sch.n_cores, num_swdge_queues=4)

    uf = nc.dram_tensor("uf", [N, P], tbl_dt, kind="ExternalInput").ap()
    ub = nc.dram_tensor("ub", [N, P], tbl_dt, kind="ExternalInput").ap()
    ufp = nc.dram_tensor("ufp", [P, T, P], tbl_dt,
                         kind="ExternalInput").ap()
    ubp = nc.dram_tensor("ubp", [P, T, P], tbl_dt,
                         kind="ExternalInput").ap()
    idx_d = nc.dram_tensor("idx", [128, sch.idx_cols], mybir.dt.int16,
                           kind="ExternalInput").ap()
    pos_d = nc.dram_tensor("pos", [P, sch.pos_cols], m_dt,
                           kind="ExternalInput").ap()
    dinvf_d = nc.dram_tensor("dinvf", [P, T], f32, kind="ExternalInput").ap()
    dinvb_d = nc.dram_tensor("dinvb", [P, T], f32, kind="ExternalInput").ap()
    wf_d = nc.dram_tensor("wf", [P, P], m_dt, kind="ExternalInput").ap()
    wb_d = nc.dram_tensor("wb", [P, P], m_dt, kind="ExternalInput").ap()
    bias_d = nc.dram_tensor("bias", [P, P], f32, kind="ExternalInput").ap()
    iota_d = nc.dram_tensor("iota", [P, kmax, 64], m_dt,
                            kind="ExternalInput").ap()
    ident_d = nc.dram_tensor("ident", [P, P], m_dt,
                             kind="ExternalInput").ap()
    out_d = nc.dram_tensor("out", [P, T, P], f32, kind="ExternalOutput").ap()

    lim = min(HALF, N)
    tables = [uf[0:lim, :], uf[lim:N, :], ub[0:lim, :], ub[lim:N, :]]

    with tile.TileContext(nc) as tc, ExitStack() as ctx:
        const = ctx.enter_context(tc.tile_pool(name="const", bufs=1))

        def load_const(shape, dt, dram_ap, tag):
            t = const.tile(shape, dtype=dt, tag=tag)
            nc.sync.dma_start(out=t[:], in_=dram_ap)
            return t

        idx_sb = load_const([128, sch.idx_cols], mybir.dt.int16, idx_d,
                            "idx")
        pos_sb = load_const([P, sch.pos_cols], m_dt, pos_d, "pos")
        dinvf_sb = load_const([P, T], f32, dinvf_d, "dinvf")
        dinvb_sb = load_const([P, T], f32, dinvb_d, "dinvb")
        wf_sb = load_const([P, P], m_dt, wf_d, "wf")
        wb_sb = load_const([P, P], m_dt, wb_d, "wb")
        bias_sb = load_const([P, P], f32, bias_d, "bias")
        iota_sb = load_const([P, kmax, 64], m_dt, iota_d, "iota")
        ident_sb = load_const([P, P], m_dt, ident_d, "ident")

        gpools = [ctx.enter_context(tc.tile_pool(name=f"g{s}", bufs=2))
                  for s in range(4)]
        spool = ctx.enter_context(tc.tile_pool(name="selfp", bufs=2))
        mpool = ctx.enter_context(tc.tile_pool(name="m", bufs=4))
        aggp = ctx.enter_context(tc.tile_pool(name="aggp", bufs=2,
                                              space="PSUM"))
        outp = ctx.enter_context(tc.tile_pool(name="outp", bufs=2,
                                              space="PSUM"))
        sbp = ctx.enter_context(tc.tile_pool(name="sbp", bufs=4))
        obp = ctx.enter_context(tc.tile_pool(name="obp", bufs=2))

        qload = [0, 0, 0, 0]
        for _dup in range(dup):
            for gi, g in enumerate(sch.groups):
                gt0 = g[0]
                ng = len(g)
                gts = []
                for s in range(4):
                    col0, n_idx = sch.idx_meta[gi][s]
                    if n_idx == 0:
                        gts.append(None)
                        continue
                    n_chunks = n_idx // P
                    gt = gpools[s].tile([P, n_chunks, P], dtype=tbl_dt,
                                        tag=f"g{s}")
                    # ~3072 idxs per dma_gather (verified OK on HW)
                    for c0 in range(0, n_chunks, 24):
                        nsub = min(24, n_chunks - c0)
                        # queue aligned to the round-robin DMASW sem lane so
                        # each lane sees a single queue (sim invariant)
                        q = (qload[0] % 8) % 4
                        qload[0] += 1
                        nc.gpsimd.dma_gather(
                            out_ap=gt[:, c0:c0 + nsub, :],
                            in_ap=tables[s],
                            idxs_ap=idx_sb[:, col0 + c0 * 8:
                                           col0 + (c0 + nsub) * 8],
                            num_idxs=nsub * P,
                            num_idxs_reg=nsub * P,
                            elem_size=P,
                            queue_num=q,
                            single_packet=False,
                        )
                    gts.append(gt)

                selfF = spool.tile([P, ng, P], dtype=tbl_dt, tag="selfF")
                nc.sync.dma_start(out=selfF[:], in_=ufp[:, gt0:gt0 + ng, :])
                selfB = spool.tile([P, ng, P], dtype=tbl_dt, tag="selfB")
                nc.sync.dma_start(out=selfB[:], in_=ubp[:, gt0:gt0 + ng, :])
                ob = obp.tile([P, ng, P], dtype=f32, tag="ob")

                for ti, t in enumerate(g):
                    # the two half-sections' pos columns are adjacent, so one
                    # 64-wide is_equal per (stream, tile) builds both halves'
                    # M; foreign lanes in shared boundary chunks read SENT
                    mts = {}
                    for s in range(4):
                        sp0 = int(sch.spans[s, t, 0, 1])
                        sp1 = int(sch.spans[s, t, 1, 1])
                        if sp0 + sp1 == 0:
                            continue
                        mt = mpool.tile([P, sp0 + sp1, 64], dtype=m_dt,
                                        tag=f"m{s}")
                        po = int(sch.pos_off2[s, t, 0])
                        nc.vector.tensor_tensor(
                            out=mt[:],
                            in0=pos_sb[:, po:po + sp0 + sp1,
                                       None].to_broadcast([P, sp0 + sp1, 64]),
                            in1=iota_sb[:, 0:sp0 + sp1, :],
                            op=mybir.AluOpType.is_equal,
                        )
                        mts[s] = mt

                    def accum(psum, s_list, self_tile):
                        parts = []
                        for s in s_list:
                            sp0 = int(sch.spans[s, t, 0, 1])
                            for h in range(2):
                                c_lo = int(sch.spans[s, t, h, 0])
                                span = int(sch.spans[s, t, h, 1])
                                if span:
                                    parts.append(
                                        (s, h, c_lo, span, sp0 * h))
                        n_tot = 1 + sum(pp[3] for pp in parts)
                        nc.tensor.matmul(out=psum[:],
                                         lhsT=self_tile[:, ti, :],
                                         rhs=ident_sb[:], start=True,
                                         stop=(n_tot == 1))
                        ci = 1
                        for s, h, c_lo, span, moff in parts:
                            ocols = psum[:, 0:64] if h == 0 else psum[:, 64:128]
                            for c in range(span):
                                nc.tensor.matmul(
                                    out=ocols,
                                    lhsT=gts[s][:, c_lo + c, :],
                                    rhs=mts[s][:, moff + c, :],
                                    start=False,
                                    stop=(ci == n_tot - 1),
                                )
                                ci += 1

                    aggf = aggp.tile([P, P], dtype=f32, tag="aggf")
                    accum(aggf, (0, 1), selfF)
                    aggb = aggp.tile([P, P], dtype=f32, tag="aggb")
                    accum(aggb, (2, 3), selfB)

                    # PSUM->SBUF copies on Act (func=Copy); fp16 out so
                    # the W GEMM runs at 1 cycle/row on PE
                    af = sbp.tile([P, P], dtype=tbl_dt, tag="af")
                    nc.scalar.copy(out=af[:], in_=aggf[:])
                    ab = sbp.tile([P, P], dtype=tbl_dt, tag="ab")
                    nc.scalar.copy(out=ab[:], in_=aggb[:])

                    outf = outp.tile([P, P], dtype=f32, tag="outf")
                    nc.tensor.matmul(out=outf[:], lhsT=af[:], rhs=wf_sb[:],
                                     start=True, stop=True)
                    outb = outp.tile([P, P], dtype=f32, tag="outb")
                    nc.tensor.matmul(out=outb[:], lhsT=ab[:], rhs=wb_sb[:],
                                     start=True, stop=True)

                    # dinv scaling on Act (func=Copy w/ per-partition scale)
                    s1 = sbp.tile([P, P], dtype=f32, tag="s1")
                    nc.scalar.mul(out=s1[:], in_=outf[:],
                                  mul=dinvf_sb[:, t:t + 1])
                    s2 = sbp.tile([P, P], dtype=f32, tag="s2")
                    nc.scalar.mul(out=s2[:], in_=outb[:],
                                  mul=dinvb_sb[:, t:t + 1])
                    s3 = sbp.tile([P, P], dtype=f32, tag="s3")
                    nc.vector.tensor_tensor(out=s3[:], in0=s1[:], in1=s2[:],
                                            op=mybir.AluOpType.add)
                    s4 = sbp.tile([P, P], dtype=f32, tag="s4")
                    nc.vector.tensor_tensor(out=s4[:], in0=s3[:],
                                            in1=bias_sb[:],
                                            op=mybir.AluOpType.add)
                    nc.vector.tensor_scalar(
                        out=ob[:, ti, :], in0=s4[:], scalar1=0.0,
                        scalar2=None, op0=mybir.AluOpType.max)

                nc.sync.dma_start(out=out_d[:, gt0:gt0 + ng, :], in_=ob[:])

    nc.compile()
    return nc


# ---------------------------------------------------------------------------
# entry point
# ---------------------------------------------------------------------------

_CACHE = {}


def run_sch(sch, trace=False, **kw):
    from concourse.bass_utils import run_bass_kernel_spmd
    key = ("prog", sch.T, sch.spans.tobytes(), sch.idx_cols,
           sch.pos_cols, str(np.dtype(sch.tbl_np)), str(np.dtype(sch.m_np)))
    if key not in _CACHE:
        _CACHE.clear()
        _CACHE[key] = build_program(sch)
    nc = _CACHE[key]
    return run_bass_kernel_spmd(
        nc, sch.in_maps, core_ids=list(range(sch.n_cores)), trace=trace, **kw)


def assemble(sch, results):
    out = np.zeros((sch.N, P), dtype=np.float32)
    for core in range(sch.n_cores):
        o = results[core]["out"]  # [128, T, 128]; slot (t, p) at o[p, t]
        rows = o.transpose(1, 0, 2).reshape(-1, P)
        sn = sch.slot_node[core]
        m = sn >= 0
        out[sn[m]] = rows[m]
    return out


def kernel(x, edge_index, W_fwd, b_fwd, W_bwd, b_bwd):
    x = np.asarray(x, dtype=np.float32)
    edge_index = np.asarray(edge_index, dtype=np.int32)
    sch = preprocess(
        x, edge_index,
        np.asarray(W_fwd, np.float32), np.asarray(b_fwd, np.float32),
        np.asarray(W_bwd, np.float32), np.asarray(b_bwd, np.float32))
    res = run_sch(sch)
    return assemble(sch, res.results)



# revision 48
# speedup vs baseline: 3.2439x; 1.8610x over previous
"""Trainium2 Bass kernel for nn_DiffusionConv (two-direction GCN conv + relu).

out = relu(gcn(x, W_fwd; src->dst) + gcn(x, W_bwd; dst->src))

Algorithm (validated against the jax reference in numpy):
  gcn(x, W; edges) = D^-1/2 (A + I) D^-1/2 x W  with D = indegree+1.
  The weight GEMM commutes with aggregation, so the device aggregates scaled
  features u = dinv * x over edges (gather rows by source, reduce into
  destination tiles), applies W to each aggregated 128-row tile, scales by
  destination dinv, adds bias, relu.

Device mapping (one SPMD program on 8 cores):
  - nodes are permuted into tiles of 128 slots; tiles sharded across cores,
    processed in groups of 8 tiles per core.
  - per (group, stream), in-edges are chunked into groups of 128; batched
    dma_gather (3072 rows per call -- large calls amortize the ~1us SWDGE
    fixed cost, and keeping total DMA-instruction count low avoids a HW
    cliff; 4 SWDGE queues aligned to the 8 round-robin DMASW sem lanes)
    pulls source rows G [128e, 128f] from the fp16 table in DRAM.
  - M[e, r] = (dst_slot[e] == r) built on DVE with one is_equal against a
    precomputed iota matrix; PE matmul agg_T += G.T @ M accumulates in PSUM.
    M is 64 columns wide: each (stream, tile) edge section is ordered
    [slots 0-63 | slots 64-127] (pos stored mod 64) and the matmuls target
    the matching PSUM column half -- halves both the DVE compare work and
    the PE matmul width. Sections are packed at their exact cross-core-max
    lengths (no rounding to 128): neighboring sections share boundary
    chunks, which get one matmul per touching section with foreign lanes
    SENT-masked, cutting gather descriptors ~16%.
  - self-loop terms use one direct DMA per group of the slot-permuted u
    tables (stored transposed [128, T, 128] in DRAM so the group slice is
    per-partition contiguous) and one identity matmul per (conv, tile).
  - per tile: PSUM->SBUF copies and per-partition dinv scaling on the
    Activation engine (all func=Copy, so no act-table reloads), fp32 GEMM
    with W on PE, add/bias/relu on DVE into a per-group output buffer,
    one output DMA per group ([128, T, 128] layout, untransposed on host).
  - dma_gather indices are int16, so each conv's edge stream is split by
    source-node half (< 32768 vs >= 32768): 4 streams total.
"""

import sys

if "/opt/trn_rl_repo" not in sys.path:
    sys.path.insert(0, "/opt/trn_rl_repo")

import numpy as np

P = 128
M_W = 32   # M matrix width (slot bucket size)
NB = P // M_W  # number of slot buckets
HALF = 32768  # int16 index limit for dma_gather
N_CORES = 8
SENT = 20000.0  # sentinel dst-slot for padded edge lanes (matches no r)


class Schedule:
    pass


def _assign_slots(cnt, n_tiles, T, n_cores, iters=30000, seed=0):
    """Assign tiles to (core, slot) minimizing sum_j sum_s ceil(max_8 / 128).

    cnt: [4, n_tiles] per-tile stream counts. Returns tile_core, tile_slot.
    """
    order = np.argsort(-cnt.sum(0), kind="stable")
    slot_tiles = [list(order[j * n_cores:(j + 1) * n_cores])
                  for j in range(T)]

    c = cnt.T  # [n_tiles, 4]

    def slot_cost(tiles):
        # sections are packed exactly, so cost = raw cross-core max sum
        return int(c[tiles].max(axis=0).sum())

    costs = [slot_cost(st) for st in slot_tiles]
    rng = np.random.default_rng(seed)
    ra = rng.integers(0, T, iters)
    rb = rng.integers(0, T, iters)
    ri = rng.integers(0, n_cores, iters)
    rj = rng.integers(0, n_cores, iters)
    for a, b, i, j in zip(ra, rb, ri, rj):
        if a == b:
            continue
        sa, sb = slot_tiles[a], slot_tiles[b]
        sa[i], sb[j] = sb[j], sa[i]
        na, nb = slot_cost(sa), slot_cost(sb)
        if na + nb < costs[a] + costs[b]:
            costs[a], costs[b] = na, nb
        else:
            sa[i], sb[j] = sb[j], sa[i]

    tile_core = np.empty(n_tiles, dtype=np.int64)
    tile_slot = np.empty(n_tiles, dtype=np.int64)
    for j, st in enumerate(slot_tiles):
        for ci, tl in enumerate(st):
            tile_core[tl] = ci
            tile_slot[tl] = j
    return tile_core, tile_slot


def preprocess(x, edge_index, W_fwd, b_fwd, W_bwd, b_bwd,
               tbl_np=np.float16, m_np=np.float16,
               group_tiles=8, n_cores=N_CORES):
    N, D = x.shape
    assert D == P
    src = edge_index[0].astype(np.int64)
    dst = edge_index[1].astype(np.int64)

    deg_f = np.bincount(dst, minlength=N) + 1.0
    deg_b = np.bincount(src, minlength=N) + 1.0
    dinv_f = (1.0 / np.sqrt(deg_f)).astype(np.float32)
    dinv_b = (1.0 / np.sqrt(deg_b)).astype(np.float32)

    u_f = (dinv_f[:, None] * x).astype(tbl_np)
    u_b = (dinv_b[:, None] * x).astype(tbl_np)

    # ---- per-node stream counts (no self-loops; they go the direct path) ----
    f_lo = np.bincount(dst[src < HALF], minlength=N)
    f_hi = np.bincount(dst[src >= HALF], minlength=N)
    b_lo = np.bincount(src[dst < HALF], minlength=N)
    b_hi = np.bincount(src[dst >= HALF], minlength=N)
    total = f_lo + f_hi + b_lo + b_hi

    n_tiles = -(-N // P)
    n_tiles = -(-n_tiles // n_cores) * n_cores
    T = n_tiles // n_cores

    # node -> tile: snake round-robin by decreasing total weight
    order = np.argsort(-total, kind="stable")
    tile_of_rank = np.arange(N) % n_tiles
    sweep = np.arange(N) // n_tiles
    odd = (sweep % 2) == 1
    tile_of_rank[odd] = n_tiles - 1 - tile_of_rank[odd]
    node_tile = np.empty(N, dtype=np.int64)
    node_tile[order] = tile_of_rank
    node_pos = np.empty(N, dtype=np.int64)
    by_tile = np.argsort(node_tile, kind="stable")
    sorted_tiles = node_tile[by_tile]
    pos_seq = np.arange(N) - np.searchsorted(sorted_tiles, sorted_tiles)
    # alternate slot halves by arrival (heaviest first) so per-half edge
    # counts stay balanced for the 64-wide M split
    pos_seq = (pos_seq % NB) * M_W + pos_seq // NB
    node_pos[by_tile] = pos_seq
    assert node_pos.max() < P

    # per-tile (stream, slot-half) counts -> tile -> (core, slot) assignment
    # balanced over all 8 columns (4 streams x 2 halves of the 64-wide M)
    cnt2 = np.zeros((4, n_tiles, NB), dtype=np.int64)
    key_sets = [dst[src < HALF], dst[src >= HALF],
                src[dst < HALF], src[dst >= HALF]]
    for i, kk in enumerate(key_sets):
        np.add.at(cnt2[i], (node_tile[kk], node_pos[kk] // M_W), 1)
    tile_core, tile_slot = _assign_slots(
        cnt2.transpose(0, 2, 1).reshape(4 * NB, n_tiles),
        n_tiles, T, n_cores)

    # ---- edge streams: (f,lo) (f,hi) (b,lo) (b,hi) ----
    # within each (stream, tile) section, slot-half 0 (pos<64) edges come
    # first so M matrices can be 64 columns wide (pos stored mod 64)
    def split(keys, gidx):
        lo = gidx < HALF
        res = []
        for mask, bse in ((lo, 0), (~lo, HALF)):
            k = keys[mask]
            g = gidx[mask] - bse
            t = node_tile[k]
            hh = node_pos[k] // M_W
            o = np.lexsort((g, hh, t))
            res.append((t[o], g[o], node_pos[k[o]]))
        return res

    streams = split(dst, src) + split(src, dst)

    # exact per-half section lengths (max across the 8 cores sharing each
    # slot, NOT rounded to chunks) -- sections share chunks at static
    # boundary offsets; boundary chunks get one matmul per touching section
    L2 = np.zeros((4, T, NB), dtype=np.int64)
    for s in range(4):
        for h in range(NB):
            per_slot = np.zeros(T, dtype=np.int64)
            np.maximum.at(per_slot, tile_slot, cnt2[s, :, h])
            L2[s, :, h] = per_slot

    # ---- static schedule layout ----
    groups = [list(range(g, min(g + group_tiles, T)))
              for g in range(0, T, group_tiles)]

    sec_off = np.zeros((4, T, NB), dtype=np.int64)  # lane off within (g,s)
    spans = np.zeros((4, T, NB, 2), dtype=np.int64)  # [s,t,h] -> c_lo, span
    pos_off2 = np.zeros((4, T, NB), dtype=np.int64)
    idx_meta = []  # per group: list of (col_start, n_idx) per stream
    c = 0
    pc = 0
    for g in groups:
        row = []
        for s in range(4):
            off = 0
            for t in g:
                for h in range(NB):
                    sec_off[s, t, h] = off
                    ln = int(L2[s, t, h])
                    if ln:
                        c_lo = off // P
                        span = -(-(off + ln) // P) - c_lo
                    else:
                        c_lo, span = 0, 0
                    spans[s, t, h] = (c_lo, span)
                    pos_off2[s, t, h] = pc
                    pc += span
                    off += ln
            n_idx = -(-off // P) * P  # pad stream tail to whole chunks
            row.append((c, n_idx))
            c += n_idx // 16
        idx_meta.append(row)
    idx_cols = max(c, 1)
    pos_cols = max(pc, 1)
    kmax = max(int(spans[..., 1].sum(-1).max()), 1)
    K = L2.sum(-1)  # informational (chunks estimate for logging)

    # ---- per-core packing ----
    slot_node = np.full((n_cores, T * P), -1, dtype=np.int64)
    g_slot = tile_slot[node_tile] * P + node_pos
    for core in range(n_cores):
        m = tile_core[node_tile] == core
        slot_node[core, g_slot[m]] = np.arange(N)[m]

    bias_sum = (b_fwd + b_bwd).astype(np.float32)
    wf = np.ascontiguousarray(W_fwd.astype(m_np))
    wb = np.ascontiguousarray(W_bwd.astype(m_np))
    bias_mat = np.ascontiguousarray(
        np.broadcast_to(bias_sum[None, :], (P, P))).astype(np.float32)
    iota_mat = np.ascontiguousarray(
        np.broadcast_to(np.arange(M_W, dtype=np.float32)[None, None, :],
                        (P, kmax, M_W))).astype(m_np)
    ident = np.eye(P, dtype=np.float32).astype(m_np)

    in_maps = []
    for core in range(n_cores):
        idx_t = np.zeros((16, idx_cols), dtype=np.int16)
        pos_t = np.full((P, pos_cols), SENT, dtype=np.float32)
        for s in range(4):
            tt, gg, pp = streams[s]
            m = tile_core[tt] == core
            t_loc = tile_slot[tt[m]]
            g_loc = gg[m]
            p_loc = pp[m]
            o = np.argsort(t_loc, kind="stable")
            t_loc, g_loc, p_loc = t_loc[o], g_loc[o], p_loc[o]
            starts = np.searchsorted(t_loc, np.arange(T))
            ends = np.searchsorted(t_loc, np.arange(T), side="right")
            for gi, g in enumerate(groups):
                col0, n_idx = idx_meta[gi][s]
                if n_idx == 0:
                    continue
                flat = np.zeros(n_idx, dtype=np.int16)
                for t in g:
                    s0, e0 = int(starts[t]), int(ends[t])
                    gl, pl = g_loc[s0:e0], p_loc[s0:e0]
                    bk = pl // M_W
                    for h in range(NB):
                        msk = bk == h
                        gv, pv = gl[msk], pl[msk] - h * M_W
                        cntc = len(gv)
                        assert cntc <= int(L2[s, t, h])
                        so = int(sec_off[s, t, h])
                        flat[so:so + cntc] = gv
                        c_lo, span = (int(spans[s, t, h, 0]),
                                      int(spans[s, t, h, 1]))
                        if span == 0:
                            continue
                        w = np.full(span * P, SENT, dtype=np.float32)
                        rel = so - c_lo * P
                        w[rel:rel + cntc] = pv
                        po = int(pos_off2[s, t, h])
                        pos_t[:, po:po + span] = w.reshape(span, P).T
                idx_t[:, col0:col0 + n_idx // 16] = flat.reshape(-1, 16).T
        pos_t = pos_t.astype(m_np)

        dvf = np.zeros((P, T), dtype=np.float32)
        dvb = np.zeros((P, T), dtype=np.float32)
        sn = slot_node[core].reshape(T, P)
        valid = sn >= 0
        dvf.T[valid] = dinv_f[sn[valid]]
        dvb.T[valid] = dinv_b[sn[valid]]

        # slot-permuted self tables (zero rows for dummy slots), stored
        # transposed [128, T, 128] so one contiguous DMA covers a group
        ufp = np.zeros((T * P, P), dtype=tbl_np)
        ubp = np.zeros((T * P, P), dtype=tbl_np)
        snf = slot_node[core]
        vm = snf >= 0
        ufp[vm] = u_f[snf[vm]]
        ubp[vm] = u_b[snf[vm]]
        ufpT = np.ascontiguousarray(
            ufp.reshape(T, P, P).transpose(1, 0, 2))
        ubpT = np.ascontiguousarray(
            ubp.reshape(T, P, P).transpose(1, 0, 2))

        in_maps.append({
            "uf": u_f, "ub": u_b, "ufp": ufpT, "ubp": ubpT,
            "idx": np.tile(idx_t, (8, 1)), "pos": pos_t,
            "dinvf": np.ascontiguousarray(dvf),
            "dinvb": np.ascontiguousarray(dvb),
            "wf": wf, "wb": wb, "bias": bias_mat,
            "iota": iota_mat, "ident": ident,
        })

    sch = Schedule()
    sch.T, sch.K, sch.groups = T, K, groups
    sch.spans, sch.pos_off2 = spans, pos_off2
    sch.idx_meta = idx_meta
    sch.idx_cols, sch.pos_cols = idx_cols, pos_cols
    sch.in_maps = in_maps
    sch.slot_node = slot_node
    sch.kmax = kmax
    sch.N, sch.n_cores = N, n_cores
    sch.tbl_np, sch.m_np = tbl_np, m_np
    return sch


# ---------------------------------------------------------------------------
# device program
# ---------------------------------------------------------------------------

def build_program(sch, debug=False, dup=1):
    from contextlib import ExitStack
    import concourse.mybir as mybir
    import concourse.tile as tile
    from concourse import bacc

    tbl_dt = mybir.dt.from_np(np.dtype(sch.tbl_np))
    m_dt = mybir.dt.from_np(np.dtype(sch.m_np))
    f32 = mybir.dt.float32
    T, K, kmax = sch.T, sch.K, sch.kmax
    N = sch.N

    nc = bacc.Bacc("TRN2", target_bir_lowering=False, debug=debug,
                   num_devices=sch.n_cores, num_swdge_queues=4)

    uf = nc.dram_tensor("uf", [N, P], tbl_dt, kind="ExternalInput").ap()
    ub = nc.dram_tensor("ub", [N, P], tbl_dt, kind="ExternalInput").ap()
    ufp = nc.dram_tensor("ufp", [P, T, P], tbl_dt,
                         kind="ExternalInput").ap()
    ubp = nc.dram_tensor("ubp", [P, T, P], tbl_dt,
                         kind="ExternalInput").ap()
    idx_d = nc.dram_tensor("idx", [128, sch.idx_cols], mybir.dt.int16,
                           kind="ExternalInput").ap()
    pos_d = nc.dram_tensor("pos", [P, sch.pos_cols], m_dt,
                           kind="ExternalInput").ap()
    dinvf_d = nc.dram_tensor("dinvf", [P, T], f32, kind="ExternalInput").ap()
    dinvb_d = nc.dram_tensor("dinvb", [P, T], f32, kind="ExternalInput").ap()
    wf_d = nc.dram_tensor("wf", [P, P], m_dt, kind="ExternalInput").ap()
    wb_d = nc.dram_tensor("wb", [P, P], m_dt, kind="ExternalInput").ap()
    bias_d = nc.dram_tensor("bias", [P, P], f32, kind="ExternalInput").ap()
    iota_d = nc.dram_tensor("iota", [P, kmax, M_W], m_dt,
                            kind="ExternalInput").ap()
    ident_d = nc.dram_tensor("ident", [P, P], m_dt,
                             kind="ExternalInput").ap()
    out_d = nc.dram_tensor("out", [P, T, P], f32, kind="ExternalOutput").ap()

    lim = min(HALF, N)
    tables = [uf[0:lim, :], uf[lim:N, :], ub[0:lim, :], ub[lim:N, :]]

    with tile.TileContext(nc) as tc, ExitStack() as ctx:
        const = ctx.enter_context(tc.tile_pool(name="const", bufs=1))

        def load_const(shape, dt, dram_ap, tag):
            t = const.tile(shape, dtype=dt, tag=tag)
            nc.sync.dma_start(out=t[:], in_=dram_ap)
            return t

        idx_sb = load_const([128, sch.idx_cols], mybir.dt.int16, idx_d,
                            "idx")
        pos_sb = load_const([P, sch.pos_cols], m_dt, pos_d, "pos")
        dinvf_sb = load_const([P, T], f32, dinvf_d, "dinvf")
        dinvb_sb = load_const([P, T], f32, dinvb_d, "dinvb")
        wf_sb = load_const([P, P], m_dt, wf_d, "wf")
        wb_sb = load_const([P, P], m_dt, wb_d, "wb")
        bias_sb = load_const([P, P], f32, bias_d, "bias")
        iota_sb = load_const([P, kmax, M_W], m_dt, iota_d, "iota")
        ident_sb = load_const([P, P], m_dt, ident_d, "ident")

        gpools = [ctx.enter_context(tc.tile_pool(name=f"g{s}", bufs=2))
                  for s in range(4)]
        spool = ctx.enter_context(tc.tile_pool(name="selfp", bufs=2))
        mpool = ctx.enter_context(tc.tile_pool(name="m", bufs=4))
        aggp = ctx.enter_context(tc.tile_pool(name="aggp", bufs=2,
                                              space="PSUM"))
        outp = ctx.enter_context(tc.tile_pool(name="outp", bufs=2,
                                              space="PSUM"))
        sbp = ctx.enter_context(tc.tile_pool(name="sbp", bufs=4))
        obp = ctx.enter_context(tc.tile_pool(name="obp", bufs=2))

        qload = [0, 0, 0, 0]
        for _dup in range(dup):
            for gi, g in enumerate(sch.groups):
                gt0 = g[0]
                ng = len(g)
                gts = []
                for s in range(4):
                    col0, n_idx = sch.idx_meta[gi][s]
                    if n_idx == 0:
                        gts.append(None)
                        continue
                    n_chunks = n_idx // P
                    gt = gpools[s].tile([P, n_chunks, P], dtype=tbl_dt,
                                        tag=f"g{s}")
                    # ~3072 idxs per dma_gather (verified OK on HW)
                    for c0 in range(0, n_chunks, 24):
                        nsub = min(24, n_chunks - c0)
                        # queue aligned to the round-robin DMASW sem lane so
                        # each lane sees a single queue (sim invariant)
                        q = (qload[0] % 8) % 4
                        qload[0] += 1
                        nc.gpsimd.dma_gather(
                            out_ap=gt[:, c0:c0 + nsub, :],
                            in_ap=tables[s],
                            idxs_ap=idx_sb[:, col0 + c0 * 8:
                                           col0 + (c0 + nsub) * 8],
                            num_idxs=nsub * P,
                            num_idxs_reg=nsub * P,
                            elem_size=P,
                            queue_num=q,
                            single_packet=False,
                        )
                    gts.append(gt)

                selfF = spool.tile([P, ng, P], dtype=tbl_dt, tag="selfF")
                nc.sync.dma_start(out=selfF[:], in_=ufp[:, gt0:gt0 + ng, :])
                selfB = spool.tile([P, ng, P], dtype=tbl_dt, tag="selfB")
                nc.sync.dma_start(out=selfB[:], in_=ubp[:, gt0:gt0 + ng, :])
                ob = obp.tile([P, ng, P], dtype=f32, tag="ob")

                for ti, t in enumerate(g):
                    # all NB bucket-sections' pos columns are adjacent, so
                    # one M_W-wide is_equal per (stream, tile) builds every
                    # bucket's M; foreign lanes in shared chunks read SENT
                    mts = {}
                    for s in range(4):
                        spsum = int(sch.spans[s, t, :, 1].sum())
                        if spsum == 0:
                            continue
                        mt = mpool.tile([P, spsum, M_W], dtype=m_dt,
                                        tag=f"m{s}")
                        po = int(sch.pos_off2[s, t, 0])
                        nc.vector.tensor_tensor(
                            out=mt[:],
                            in0=pos_sb[:, po:po + spsum,
                                       None].to_broadcast([P, spsum, M_W]),
                            in1=iota_sb[:, 0:spsum, :],
                            op=mybir.AluOpType.is_equal,
                        )
                        mts[s] = mt

                    def accum(psum, s_list, self_tile):
                        parts = []
                        for s in s_list:
                            moff = 0
                            for h in range(NB):
                                c_lo = int(sch.spans[s, t, h, 0])
                                span = int(sch.spans[s, t, h, 1])
                                if span:
                                    parts.append((s, h, c_lo, span, moff))
                                moff += span
                        n_tot = 1 + sum(pp[3] for pp in parts)
                        nc.tensor.matmul(out=psum[:],
                                         lhsT=self_tile[:, ti, :],
                                         rhs=ident_sb[:], start=True,
                                         stop=(n_tot == 1))
                        ci = 1
                        for s, h, c_lo, span, moff in parts:
                            ocols = psum[:, h * M_W:(h + 1) * M_W]
                            for c in range(span):
                                nc.tensor.matmul(
                                    out=ocols,
                                    lhsT=gts[s][:, c_lo + c, :],
                                    rhs=mts[s][:, moff + c, :],
                                    start=False,
                                    stop=(ci == n_tot - 1),
                                )
                                ci += 1

                    aggf = aggp.tile([P, P], dtype=f32, tag="aggf")
                    accum(aggf, (0, 1), selfF)
                    aggb = aggp.tile([P, P], dtype=f32, tag="aggb")
                    accum(aggb, (2, 3), selfB)

                    # PSUM->SBUF copies on Act (func=Copy); fp16 out so
                    # the W GEMM runs at 1 cycle/row on PE
                    af = sbp.tile([P, P], dtype=tbl_dt, tag="af")
                    nc.scalar.copy(out=af[:], in_=aggf[:])
                    ab = sbp.tile([P, P], dtype=tbl_dt, tag="ab")
                    nc.scalar.copy(out=ab[:], in_=aggb[:])

                    outf = outp.tile([P, P], dtype=f32, tag="outf")
                    nc.tensor.matmul(out=outf[:], lhsT=af[:], rhs=wf_sb[:],
                                     start=True, stop=True)
                    outb = outp.tile([P, P], dtype=f32, tag="outb")
                    nc.tensor.matmul(out=outb[:], lhsT=ab[:], rhs=wb_sb[:],
                                     start=True, stop=True)

                    # dinv scaling on Act (func=Copy w/ per-partition scale)
                    s1 = sbp.tile([P, P], dtype=f32, tag="s1")
                    nc.scalar.mul(out=s1[:], in_=outf[:],
                                  mul=dinvf_sb[:, t:t + 1])
                    s2 = sbp.tile([P, P], dtype=f32, tag="s2")
                    nc.scalar.mul(out=s2[:], in_=outb[:],
                                  mul=dinvb_sb[:, t:t + 1])
                    s3 = sbp.tile([P, P], dtype=f32, tag="s3")
                    nc.vector.tensor_tensor(out=s3[:], in0=s1[:], in1=s2[:],
                                            op=mybir.AluOpType.add)
                    s4 = sbp.tile([P, P], dtype=f32, tag="s4")
                    nc.vector.tensor_tensor(out=s4[:], in0=s3[:],
                                            in1=bias_sb[:],
                                            op=mybir.AluOpType.add)
                    nc.vector.tensor_scalar(
                        out=ob[:, ti, :], in0=s4[:], scalar1=0.0,
                        scalar2=None, op0=mybir.AluOpType.max)

                nc.sync.dma_start(out=out_d[:, gt0:gt0 + ng, :], in_=ob[:])

    nc.compile()
    return nc


# ---------------------------------------------------------------------------
# entry point
# ---------------------------------------------------------------------------

_CACHE = {}


def run_sch(sch, trace=False, **kw):
    from concourse.bass_utils import run_bass_kernel_spmd
    key = ("prog", sch.T, sch.spans.tobytes(), sch.idx_cols,
           sch.pos_cols, str(np.dtype(sch.tbl_np)), str(np.dtype(sch.m_np)))
    if key not in _CACHE:
        _CACHE.clear()
        _CACHE[key] = build_program(sch)
    nc = _CACHE[key]
    return run_bass_kernel_spmd(
        nc, sch.in_maps, core_ids=list(range(sch.n_cores)), trace=trace, **kw)


def assemble(sch, results):
    out = np.zeros((sch.N, P), dtype=np.float32)
    for core in range(sch.n_cores):
        o = results[core]["out"]  # [128, T, 128]; slot (t, p) at o[p, t]
        rows = o.transpose(1, 0, 2).reshape(-1, P)
        sn = sch.slot_node[core]
        m = sn >= 0
        out[sn[m]] = rows[m]
    return out


def kernel(x, edge_index, W_fwd, b_fwd, W_bwd, b_bwd):
    x = np.asarray(x, dtype=np.float32)
    edge_index = np.asarray(edge_index, dtype=np.int32)
    sch = preprocess(
        x, edge_index,
        np.asarray(W_fwd, np.float32), np.asarray(b_fwd, np.float32),
        np.asarray(W_bwd, np.float32), np.asarray(b_bwd, np.float32))
    res = run_sch(sch)
    return assemble(sch, res.results)

